# revision 20
# baseline (speedup 1.0000x reference)
"""Trainium2 Bass kernel for nn_EDTransformer (encoder-decoder transformer).

Sharding: 8 cores = 4 batch items x 2 sequence halves.
 - Each core owns (item b, half h): Q/scores/AV/Wo/MLP/LN for its 256 local
   positions, K/V redundantly for the full 512 positions.
 - One 2-core AllGather of fp16 activations per layer boundary.
 - Unembedding position-sharded (full vocab per core), softmax denominator
   via fused activation accumulate.

Precision plan:
 - Q/K projections in fp8e4 DoubleRow matmuls (2x PE rate): their
   quantization noise only jitters softmax scores, which averages out.
 - Everything on the value path (V fill, exp, AV, deno, y, Wo) plus MLP,
   LN and unembed stays fp16 with fp32 PSUM accumulation.
 - Residual adds are injected into the Wo/W2 PSUM by an identity matmul
   (frees the vector engine; PSUM zero-region seeded by even-dt start).
"""
import sys

sys.path.insert(0, '/opt/trn_rl_repo')
import numpy as np

import concourse.bacc as bacc
import concourse.tile as tile
import concourse.mybir as mybir
from concourse.bass_utils import run_bass_kernel_spmd

DT = mybir.dt
F16 = DT.float16
F32 = DT.float32
F8 = DT.float8e4
DR = mybir.MatmulPerfMode.DoubleRow
AF = mybir.ActivationFunctionType
ALU = mybir.AluOpType

N_CORES = 8
P = 128
DE = 1024          # model dim        (8 ptiles)
KO = DE // P       # 8
DMLP = 4096        # mlp dim          (32 ptiles)
MO = DMLP // P     # 32
H = 16             # heads
DA = 64            # attn dim per head
L = 512            # sequence length
LL = 256           # local positions per core
NV = 32000
UC = 500           # unembed vocab chunk (moving N)
UNC = NV // UC     # 64 chunks
LENC = 2
LDEC = 2
EPS = 1e-5

WS = 32.0                     # fp8 weight scale for Wq/Wk
EXPS = 1.0 / (WS * WS * 8.0)  # exp scale (folds q8*k8 scale + sqrt(da))

PAIR_GROUPS = [[0, 1], [2, 3], [4, 5], [6, 7]]

_CACHE = {}


# ----------------------------------------------------------------------------
# device program
# ----------------------------------------------------------------------------

def _attn(nc, tc, pools, pres, e16res, qin8, kv8, kv16, wq_t, wk_t,
          wv_dram, wo_dram, mask16, qs, name):
    """One multi-head attention block. Leaves pres [128, 8, 256] f32 psum
    holding attn_out + residual.

    qin8  : [128, KO, LL] fp8  local stream (query input)
    kv8   : [128, KO, L]  fp8  full-seq stream (key input)
    kv16  : [128, KO, L]  fp16 full-seq stream (value input)
    wq_t/wk_t : SBUF fp8 weight tiles [128, KO, 1024] (x32 scaled)
    wv_dram/wo_dram : fp16 dram APs [128, KO, 1024]
    mask16: [128, 4, LL] fp16 sbuf tile or None
    """
    sb = pools['att']
    pA = pools['pA']
    KT = L // P  # 4 kz tiles

    # ---- residual identity injection (start=True on even dt pre-zeroes
    # the 2KB psum zero-region covering the odd partner as well).
    for dt in range(KO):
        nc.tensor.matmul(pres[:, dt, :], pools['diag1'][:, :],
                         e16res[:, dt, :], start=(dt % 2 == 0), stop=False,
                         skip_group_check=True)

    # prefetch V weights (used after Q/K fills)
    wvts = []
    for nch in range(2):
        wvt = pools['wvp'].tile([P, KO, 512], F16, tag='wvt')
        qs[nch % 2].dma_start(wvt[:], wv_dram[:, :, nch * 512:(nch + 1) * 512])
        wvts.append(wvt)

    # ---- q16: [128(2h x 64a), pr, LL]  (fp8 DR matmul, fp16 result)
    q16 = sb.tile([P, KO, LL], F16, tag='q16')
    for pr in range(KO):
        ps = pA.tile([P, L], F32, tag='pA')
        for j in range(KO // 2):
            nc.tensor.matmul(ps[:, :LL], wq_t[:, 2 * j:2 * j + 2,
                                              pr * P:(pr + 1) * P],
                             qin8[:, 2 * j:2 * j + 2, :],
                             start=(j == 0), stop=(j == 3), perf_mode=DR)
        nc.scalar.activation(q16[:, pr, :], ps[:, :LL], AF.Copy)
    # ---- k16: [128, pr, L]
    k16 = sb.tile([P, KO, L], F16, tag='k16')
    for pr in range(KO):
        ps = pA.tile([P, L], F32, tag='pA')
        for j in range(KO // 2):
            nc.tensor.matmul(ps[:], wk_t[:, 2 * j:2 * j + 2,
                                         pr * P:(pr + 1) * P],
                             kv8[:, 2 * j:2 * j + 2, :],
                             start=(j == 0), stop=(j == 3), perf_mode=DR)
        nc.vector.tensor_copy(k16[:, pr, :], ps[:])
    # ---- vT: [128(kz), kt, 1024(h*64)] fp16
    vt16 = sb.tile([P, KT, H * DA], F16, tag='vt16')
    for nch in range(2):
        wvt = wvts[nch]
        for kt in range(KT):
            ps = pA.tile([P, L], F32, tag='pA')
            for k in range(KO):
                nc.tensor.matmul(ps[:], kv16[:, k, kt * P:(kt + 1) * P],
                                 wvt[:, k, :],
                                 start=(k == 0), stop=(k == KO - 1))
            nc.vector.tensor_copy(vt16[:, kt, nch * 512:(nch + 1) * 512],
                                  ps[:])

    # prefetch Wo chunks (consumed after the head loop)
    wots = []
    for c in range(4):
        wot = pools['wop'].tile([P, KO, 2 * P], F16, tag='wot')
        qs[c % 2].dma_start(wot[:], wo_dram[:, :, c * 2 * P:(c + 1) * 2 * P])
        wots.append(wot)

    # ---- per head pair: scores -> exp -> deno -> AV -> y16
    y16 = sb.tile([P, KO, LL], F16, tag='y16')
    for p in range(KO):
        hA = 2 * p
        # expp [128(kz), kt, hh, q] fp16 for this pair
        expp = sb.tile([P, KT, 2, LL], F16, tag='expp', bufs=2)
        for hh in range(2):
            h = hA + hh
            pr, hp = h // 2, (h % 2) * DA
            for t in range(2):
                ps = pA.tile([P, L], F32, tag='pA')
                for kt in (2 * t, 2 * t + 1):
                    nc.tensor.matmul(
                        ps[:, (kt - 2 * t) * LL:(kt - 2 * t + 1) * LL],
                        k16[hp:hp + DA, pr, kt * P:(kt + 1) * P],
                        q16[hp:hp + DA, pr, :],
                        start=True, stop=True)
                nc.scalar.activation(
                    expp[:, 2 * t:2 * t + 2, hh, :],
                    ps[:].rearrange('p (a b) -> p a b', a=2),
                    AF.Exp, scale=EXPS)
            if mask16 is not None:
                nc.vector.tensor_tensor(
                    expp[:, :, hh, :], expp[:, :, hh, :],
                    mask16[:], ALU.mult)
        # deno: ones-matmul -> [128, (hh, q)]
        pd = pA.tile([P, L], F32, tag='pA')
        for kt in range(KT):
            nc.tensor.matmul(pd[:], pools['ones16'][:, :],
                             expp[:, kt, :, :],
                             start=(kt == 0), stop=(kt == KT - 1))
        ysc = pools['yscp'].tile([P, 2 * LL], F32, tag='ysc')
        nc.vector.reciprocal_approx_fast(ysc[:], pd[:])
        # AV, pair-packed via column tiling
        pav = pA.tile([P, L], F32, tag='pA')
        for hh in range(2):
            hcol = (hA + hh) * DA
            for kt in range(KT):
                nc.tensor.matmul(
                    pav[hh * DA:(hh + 1) * DA, :LL],
                    vt16[:, kt, hcol:hcol + DA],
                    expp[:, kt, hh, :],
                    start=(kt == 0), stop=(kt == KT - 1),
                    tile_position=(0, hh * DA))
        nc.vector.tensor_tensor(y16[:DA, p, :], pav[:DA, :LL],
                                ysc[:DA, 0:LL], ALU.mult)
        nc.vector.tensor_tensor(y16[DA:, p, :], pav[DA:, :LL],
                                ysc[DA:, LL:2 * LL], ALU.mult)

    # ---- Wo accumulate into pres (start=False: id-matmul zeroed/seeded)
    for c in range(4):
        wot = wots[c]
        for di in range(2):
            dt = 2 * c + di
            for k in range(KO):
                nc.tensor.matmul(pres[:, dt, :],
                                 wot[:, k, di * P:(di + 1) * P],
                                 y16[:, k, :],
                                 start=False, stop=(k == KO - 1),
                                 skip_group_check=True)
    tp = pools.get('tapfn')
    if tp:
        tp(f'{name}_q', q16); tp(f'{name}_k', k16); tp(f'{name}_vt', vt16)
        tp(f'{name}_y', y16)


def _mlp(nc, tc, pools, pres, e16res, w1_dram, w2_dram, qs, name):
    """MLP block (fp16). Leaves pres holding mlp_out + residual."""
    pA = pools['pA']
    for dt in range(KO):
        nc.tensor.matmul(pres[:, dt, :], pools['diag1'][:, :],
                         e16res[:, dt, :], start=(dt % 2 == 0), stop=False,
                         skip_group_check=True)
    h16 = pools['mlp'].tile([P, MO, LL], F16, tag='h16')
    for c in range(8):
        w1t = pools['w1p'].tile([P, KO, 4 * P], F16, tag='w1t')
        qs[c % 2].dma_start(w1t[:], w1_dram[:, :, c * 512:(c + 1) * 512])
        for m2 in range(2):   # 2 mt per psum tile
            ps = pA.tile([P, L], F32, tag='pA')
            for mi in range(2):
                for k in range(KO):
                    nc.tensor.matmul(ps[:, mi * LL:(mi + 1) * LL],
                                     w1t[:, k, (2 * m2 + mi) * P:
                                         (2 * m2 + mi + 1) * P],
                                     e16res[:, k, :],
                                     start=(k == 0), stop=(k == KO - 1))
            nc.scalar.activation(h16[:, 4 * c + 2 * m2:4 * c + 2 * m2 + 2, :],
                                 ps[:].rearrange('p (a b) -> p a b', a=2),
                                 AF.Relu)
    for c in range(8):
        w2t = pools['w2p'].tile([P, 4, KO * P], F16, tag='w2t')
        qs[c % 2].dma_start(w2t[:], w2_dram[:, c * 4:(c + 1) * 4, :])
        for j in range(4):
            for dt in range(KO):
                nc.tensor.matmul(pres[:, dt, :],
                                 w2t[:, j, dt * P:(dt + 1) * P],
                                 h16[:, c * 4 + j, :],
                                 start=False,
                                 stop=(c == 7 and j == 3),
                                 skip_group_check=True)


def _ln(nc, tc, pools, pres, e16out, e8out, name):
    """Layernorm from psum pres -> fp16 (+fp8) stream."""
    pA = pools['pA']
    lnp = pools['lnp']
    stat = pools['stat']
    pre16 = lnp.tile([P, KO, LL], F16, tag='pre16')
    for dt in range(KO):
        nc.scalar.activation(pre16[:, dt, :], pres[:, dt, :], AF.Copy)
    sq16 = lnp.tile([P, KO, LL], F16, tag='sq16')
    nc.vector.tensor_tensor(sq16[:], pre16[:], pre16[:], ALU.mult)
    pss = pA.tile([P, L], F32, tag='pA')
    for k in range(KO):
        nc.tensor.matmul(pss[:, :LL], pools['ones16'][:, :], pre16[:, k, :],
                         start=(k == 0), stop=(k == KO - 1))
    psq = pA.tile([P, L], F32, tag='pA')
    for k in range(KO):
        nc.tensor.matmul(psq[:, :LL], pools['ones16'][:, :], sq16[:, k, :],
                         start=(k == 0), stop=(k == KO - 1))
    # var = Q/1023 - S^2/(1024*1023); inv = 1/sqrt(var+eps); nm = -S/1024*inv
    s2 = stat.tile([P, LL], F32, tag='s2')
    nc.scalar.activation(s2[:], pss[:, :LL], AF.Square)
    var = stat.tile([P, LL], F32, tag='var')
    nc.vector.tensor_scalar(var[:], s2[:], 1.0 / (1024.0 * 1023.0), None,
                            ALU.mult)
    q1 = stat.tile([P, LL], F32, tag='q1')
    nc.vector.tensor_scalar(q1[:], psq[:, :LL], 1.0 / 1023.0, None, ALU.mult)
    nc.vector.tensor_tensor(var[:], q1[:], var[:], ALU.subtract)
    std = stat.tile([P, LL], F32, tag='std')
    nc.scalar.activation(std[:], var[:], AF.Sqrt, bias=pools['eps128'])
    inv = stat.tile([P, LL], F32, tag='inv')
    nc.vector.reciprocal_approx_fast(inv[:], std[:])
    nm = stat.tile([P, LL], F32, tag='nm')
    nc.vector.tensor_tensor(nm[:], pss[:, :LL], inv[:], ALU.mult)
    nc.vector.tensor_scalar(nm[:], nm[:], -1.0 / 1024.0, None, ALU.mult)
    nc.vector.tensor_tensor(
        e16out[:], pre16[:],
        inv[:, None, :].to_broadcast((P, KO, LL)), ALU.mult)
    nc.vector.tensor_tensor(
        e16out[:], e16out[:],
        nm[:, None, :].to_broadcast((P, KO, LL)), ALU.add)
    if e8out is not None:
        nc.vector.tensor_copy(e8out[:], e16out[:])
    tp = pools.get('tapfn')
    if tp:
        tp(f'{name}_out', e16out)


def _allgather_pair(nc, tc, pools, e16loc, full16, full8, agin, agout, tag):
    """e16loc [128, KO, LL] -> pair AllGather -> full16/full8 [128, KO, L]."""
    nc.gpsimd.dma_start(agin[:], e16loc[:])
    nc.gpsimd.collective_compute(
        "AllGather", ALU.bypass,
        ins=[agin[:]], outs=[agout[:]],
        replica_groups=PAIR_GROUPS)
    nc.gpsimd.dma_start(
        full16[:].rearrange('ki ko (r p) -> ki ko r p', r=2),
        agout[:].rearrange('r ki ko p -> ki ko r p'))
    nc.vector.tensor_copy(full8[:], full16[:])


def build_program(taps=()):
    taps = set(taps)
    nc = bacc.Bacc("TRN2", target_bir_lowering=False, debug=False,
                   num_devices=N_CORES)

    # ---- dram inputs ----
    din = {}
    def dram_in(nm, shape, dt=F16):
        din[nm] = nc.dram_tensor(nm, list(shape), dt, kind="ExternalInput")
        return din[nm]

    z0f16 = dram_in('z0_full16', [P, KO, L])
    x0f16 = dram_in('x0_full16', [P, KO, L])
    z0f8 = dram_in('z0_full8', [P, KO, L], F8)
    x0f8 = dram_in('x0_full8', [P, KO, L], F8)
    z0l8 = dram_in('z0_loc8', [P, KO, LL], F8)
    x0l8 = dram_in('x0_loc8', [P, KO, LL], F8)
    z0l16 = dram_in('z0_loc16', [P, KO, LL])
    x0l16 = dram_in('x0_loc16', [P, KO, LL])
    mask_self = dram_in('mask_self16', [P, 4, LL])
    diag1_d = dram_in('diag1', [P, P])
    for pfx, nl in (('enc', LENC), ('dec', LDEC)):
        for w in ('wq8', 'wk8'):
            dram_in(f'{pfx}_{w}', [nl, P, KO, DE], F8)
        for w in ('wvT', 'woT'):
            dram_in(f'{pfx}_{w}', [nl, P, KO, DE])
        dram_in(f'{pfx}_w1T', [nl, P, KO, DMLP])
        dram_in(f'{pfx}_w2T', [nl, P, MO, DE])
    wuT = dram_in('wuT', [P, KO, NV])

    # output: [pt, ki, vocab] fp16 UNNORMALIZED exp(logits),
    # positions = h*256 + pt*128 + ki; deno_d = softmax denominators.
    outp = nc.dram_tensor('outp', [2, P, NV], F16, kind="ExternalOutput")
    deno_d = nc.dram_tensor('deno_out', [P, 2], F32, kind="ExternalOutput")

    # internal dram for pair collectives
    agin = nc.dram_tensor('agin', [P, KO, LL], F16)
    agout = nc.dram_tensor('agout', [2, P, KO, LL], F16)

    import contextlib
    with tile.TileContext(nc) as tc, contextlib.ExitStack() as octx:
        const = octx.enter_context(tc.tile_pool(name='const', bufs=1))
        ones16 = const.tile([P, P], F16)
        nc.vector.memset(ones16[:], 1.0)
        eps128 = const.tile([P, 1], F32)
        nc.vector.memset(eps128[:], EPS)
        diag1 = const.tile([P, P], F16)
        nc.sync.dma_start(diag1[:], diag1_d[:])
        msk16 = const.tile([P, 4, LL], F16)
        nc.sync.dma_start(msk16[:], mask_self[:])
        xu = const.tile([P, KO, LL], F16)

        # ================= layer phase =================
        with contextlib.ExitStack() as ctx:
            stream = ctx.enter_context(tc.tile_pool(name='stream', bufs=1))
            att = ctx.enter_context(tc.tile_pool(name='att', bufs=1))
            mlpp = ctx.enter_context(tc.tile_pool(name='mlpp', bufs=1))
            lnp = ctx.enter_context(tc.tile_pool(name='lnp', bufs=1))
            stat = ctx.enter_context(tc.tile_pool(name='stat', bufs=1))
            yscp = ctx.enter_context(tc.tile_pool(name='yscp', bufs=2))
            watp = ctx.enter_context(tc.tile_pool(name='watp', bufs=1))
            wvp = ctx.enter_context(tc.tile_pool(name='wvp', bufs=2))
            wop = ctx.enter_context(tc.tile_pool(name='wop', bufs=4))
            w1p = ctx.enter_context(tc.tile_pool(name='w1p', bufs=2))
            w2p = ctx.enter_context(tc.tile_pool(name='w2p', bufs=2))
            pA = ctx.enter_context(tc.tile_pool(name='pA', bufs=4,
                                                space='PSUM'))
            presp = ctx.enter_context(tc.tile_pool(name='presp', bufs=1,
                                                   space='PSUM'))

            pools = dict(att=att, mlp=mlpp, lnp=lnp, stat=stat, yscp=yscp,
                         pA=pA, wvp=wvp, wop=wop, w1p=w1p, w2p=w2p,
                         ones16=ones16, eps128=eps128[:], diag1=diag1)

            def tapfn(nm, t):
                if nm not in taps:
                    return
                d = nc.dram_tensor('tap_' + nm, list(t.shape),
                                   t.dtype, kind="ExternalOutput")
                nc.sync.dma_start(d[:], t[:])
            pools['tapfn'] = tapfn

            qs = [nc.sync, nc.scalar]

            def load_qk(pfx, l):
                wq = watp.tile([P, KO, DE], F8, tag='w_wq8')
                qs[0].dma_start(wq[:], din[f'{pfx}_wq8'][l])
                wk = watp.tile([P, KO, DE], F8, tag='w_wk8')
                qs[1].dma_start(wk[:], din[f'{pfx}_wk8'][l])
                return wq, wk

            # ======== encoder ========
            e16 = stream.tile([P, KO, LL], F16, tag='e16_a')
            nc.sync.dma_start(e16[:], z0l16[:])
            e8 = stream.tile([P, KO, LL], F8, tag='e8_a')
            nc.scalar.dma_start(e8[:], z0l8[:])
            Zfull = stream.tile([P, KO, L], F16, tag='Zfull')
            nc.sync.dma_start(Zfull[:], z0f16[:])
            Zfull8 = stream.tile([P, KO, L], F8, tag='Zfull8')
            nc.scalar.dma_start(Zfull8[:], z0f8[:])

            pres = presp.tile([P, KO, LL], F32, tag='pres')
            for l in range(LENC):
                wq, wk = load_qk('enc', l)
                _attn(nc, tc, pools, pres, e16, e8, Zfull8, Zfull,
                      wq, wk, din['enc_wvT'][l], din['enc_woT'][l],
                      None, qs, f'e{l}a')
                e16 = stream.tile([P, KO, LL], F16, tag='e16_b')
                e8 = stream.tile([P, KO, LL], F8, tag='e8_b')
                _ln(nc, tc, pools, pres, e16, e8, f'e{l}ln1')
                _mlp(nc, tc, pools, pres, e16, din['enc_w1T'][l],
                     din['enc_w2T'][l], qs, f'e{l}m')
                e16 = stream.tile([P, KO, LL], F16, tag='e16_a')
                e8 = stream.tile([P, KO, LL], F8, tag='e8_a')
                _ln(nc, tc, pools, pres, e16, e8, f'e{l}ln2')
                Zfull = stream.tile([P, KO, L], F16, tag='Zfull')
                Zfull8 = stream.tile([P, KO, L], F8, tag='Zfull8')
                _allgather_pair(nc, tc, pools, e16, Zfull, Zfull8,
                                agin, agout, f'e{l}')

            # ======== decoder ========
            e16 = stream.tile([P, KO, LL], F16, tag='e16_b')
            nc.sync.dma_start(e16[:], x0l16[:])
            e8 = stream.tile([P, KO, LL], F8, tag='e8_b')
            nc.scalar.dma_start(e8[:], x0l8[:])
            Xfull = stream.tile([P, KO, L], F16, tag='Xfull')
            nc.sync.dma_start(Xfull[:], x0f16[:])
            Xfull8 = stream.tile([P, KO, L], F8, tag='Xfull8')
            nc.scalar.dma_start(Xfull8[:], x0f8[:])

            for l in range(LDEC):
                wq, wk = load_qk('dec', l)
                _attn(nc, tc, pools, pres, e16, e8, Xfull8, Xfull,
                      wq, wk, din['dec_wvT'][l], din['dec_woT'][l],
                      msk16, qs, f'd{l}s')
                e16 = stream.tile([P, KO, LL], F16, tag='e16_a')
                e8 = stream.tile([P, KO, LL], F8, tag='e8_a')
                _ln(nc, tc, pools, pres, e16, e8, f'd{l}ln1')
                _attn(nc, tc, pools, pres, e16, e8, Zfull8, Zfull,
                      wq, wk, din['dec_wvT'][l], din['dec_woT'][l],
                      None, qs, f'd{l}c')
                e16 = stream.tile([P, KO, LL], F16, tag='e16_b')
                e8 = stream.tile([P, KO, LL], F8, tag='e8_b')
                _ln(nc, tc, pools, pres, e16, e8, f'd{l}ln2')
                _mlp(nc, tc, pools, pres, e16, din['dec_w1T'][l],
                     din['dec_w2T'][l], qs, f'd{l}m')
                if l < LDEC - 1:
                    e16 = stream.tile([P, KO, LL], F16, tag='e16_a')
                    e8 = stream.tile([P, KO, LL], F8, tag='e8_a')
                    _ln(nc, tc, pools, pres, e16, e8, f'd{l}ln3')
                    Xfull = stream.tile([P, KO, L], F16, tag='Xfull')
                    Xfull8 = stream.tile([P, KO, L], F8, tag='Xfull8')
                    _allgather_pair(nc, tc, pools, e16, Xfull, Xfull8,
                                    agin, agout, f'd{l}')
                else:
                    _ln(nc, tc, pools, pres, xu, None, f'd{l}ln3')

        # ========== unembed phase (position-sharded, no collectives) ========
        # Emits UNNORMALIZED exp(logits) per chunk (DMA'd out immediately,
        # overlapping the remaining matmuls) plus the per-position softmax
        # denominator; the host divides during assembly.
        with contextlib.ExitStack() as ctx:
            usb = ctx.enter_context(tc.tile_pool(name='usb', bufs=1))
            wup = ctx.enter_context(tc.tile_pool(name='wup', bufs=4))
            expp_ = ctx.enter_context(tc.tile_pool(name='expp_', bufs=4))
            up = ctx.enter_context(tc.tile_pool(name='up', bufs=6,
                                                space='PSUM'))
            dacc = usb.tile([P, 2, UNC], F32, tag='dacc')
            qs = [nc.sync, nc.scalar]
            for c in range(UNC):
                wut = wup.tile([P, KO, UC], F16, tag='wut')
                qs[c % 2].dma_start(wut[:], wuT[:, :, c * UC:(c + 1) * UC])
                for pt in range(2):
                    ps = up.tile([P, UC], F32, tag='ups', name=f'ups{c}{pt}')
                    for k in range(KO):
                        nc.tensor.matmul(ps[:], xu[:, k, pt * P:(pt + 1) * P],
                                         wut[:, k, :], start=(k == 0),
                                         stop=(k == KO - 1))
                    ex = expp_.tile([P, UC], F16, tag='uex')
                    nc.scalar.activation(ex[:], ps[:], AF.Exp,
                                         accum_out=dacc[:, pt, c:c + 1])
                    qs[(c + pt) % 2].dma_start(
                        outp[pt, :, c * UC:(c + 1) * UC], ex[:])
            deno = usb.tile([P, 2], F32, tag='deno')
            nc.vector.tensor_reduce(deno[:], dacc[:], mybir.AxisListType.X,
                                    ALU.add)
            nc.sync.dma_start(deno_d[:], deno[:])

    nc.compile()
    return nc


# ----------------------------------------------------------------------------
# host-side prep
# ----------------------------------------------------------------------------

def _to_kimaj(a):
    """[K, M] -> [128, K//128, M] with K = ko*128 + ki."""
    K, M = a.shape
    return np.ascontiguousarray(
        a.reshape(K // P, P, M).transpose(1, 0, 2))


def _fp8(a):
    return np.clip(a, -240.0, 240.0).astype(mybir.dt.np(F8))


def prep_inputs(inputs):
    f = lambda k: np.asarray(inputs[k], dtype=np.float32)
    We, Wp, Wu = f('We'), f('Wp'), f('Wu')
    x = np.asarray(inputs['x']).astype(np.int64)
    z = np.asarray(inputs['z']).astype(np.int64)

    shared = {}
    for pfx, nl in (('enc', LENC), ('dec', LDEC)):
        Wq, Wk, Wv = f(pfx + '_Wq'), f(pfx + '_Wk'), f(pfx + '_Wv')
        Wo, W1, W2 = f(pfx + '_Wo'), f(pfx + '_W1'), f(pfx + '_W2')
        wq, wk, wv, wo, w1, w2 = [], [], [], [], [], []
        for l in range(nl):
            qa = Wq[l].reshape(H * DA, DE).T * WS
            ka = Wk[l].reshape(H * DA, DE).T * WS
            va = Wv[l].transpose(2, 0, 1).reshape(DE, H * DA)
            wq.append(_to_kimaj(qa)); wk.append(_to_kimaj(ka))
            wv.append(_to_kimaj(va))
            wo.append(_to_kimaj(Wo[l].T))
            w1.append(_to_kimaj(W1[l].T))
            w2.append(_to_kimaj(W2[l].T))
        shared[f'{pfx}_wq8'] = _fp8(np.stack(wq))
        shared[f'{pfx}_wk8'] = _fp8(np.stack(wk))
        shared[f'{pfx}_wvT'] = np.stack(wv).astype(np.float16)
        shared[f'{pfx}_woT'] = np.stack(wo).astype(np.float16)
        shared[f'{pfx}_w1T'] = np.stack(w1).astype(np.float16)
        shared[f'{pfx}_w2T'] = np.stack(w2).astype(np.float16)

    shared['wuT'] = _to_kimaj(Wu.T).astype(np.float16)
    shared['diag1'] = np.eye(P, dtype=np.float16)

    pos = Wp[:L]  # [512, 1024]
    in_maps = []
    for c in range(N_CORES):
        b, h = c // 2, c % 2
        m = dict(shared)
        for nm, tok in (('z0', z[b]), ('x0', x[b])):
            E0 = (We[tok] + pos).T.astype(np.float32)      # [1024, 512]
            E0k = E0.reshape(KO, P, L)                     # [ko, ki, p]
            full = np.ascontiguousarray(E0k.transpose(1, 0, 2))
            m[nm + '_full16'] = full.astype(np.float16)
            m[nm + '_full8'] = _fp8(full)
            loc = np.ascontiguousarray(
                E0k[:, :, h * LL:(h + 1) * LL].transpose(1, 0, 2))
            m[nm + '_loc16'] = loc.astype(np.float16)
            m[nm + '_loc8'] = _fp8(loc)
        kglob = np.arange(L)[:, None]
        qglob = (h * LL + np.arange(LL))[None, :]
        msk = (kglob <= qglob).astype(np.float16)          # [512, 256]
        m['mask_self16'] = np.ascontiguousarray(
            msk.reshape(4, P, LL).transpose(1, 0, 2))
        in_maps.append(m)
    return in_maps


def assemble(results):
    """results: per-core dicts with 'outp' [2, 128, NV] fp16 (unnormalized
    exp of logits) and 'deno_out' [128, 2] f32 softmax denominators."""
    out = np.empty((4, NV, L), dtype=np.float32)
    for c, r in enumerate(results):
        b, h = c // 2, c % 2
        o = r['outp'].astype(np.float32).reshape(LL, NV)   # [pos, vocab]
        deno = r['deno_out'].T.reshape(LL, 1)              # [pos, 1]
        out[b, :, h * LL:(h + 1) * LL] = (o / deno).T
    return out


def run(inputs, trace=False, taps=(), trace_kwargs=None):
    key = ('prog', tuple(sorted(taps)))
    if key not in _CACHE:
        _CACHE[key] = build_program(taps=taps)
    nc = _CACHE[key]
    in_maps = prep_inputs(inputs)
    res = run_bass_kernel_spmd(nc, in_maps, list(range(N_CORES)),
                               trace=trace, **(trace_kwargs or {}))
    return res


def kernel(**inputs):
    res = run(inputs, trace=False)
    return assemble(res.results)


# revision 24
# speedup vs baseline: 1.0040x; 1.0040x over previous
"""Trainium2 Bass kernel for nn_EDTransformer (encoder-decoder transformer).

Sharding: 8 cores = 4 batch items x 2 sequence halves.
 - Each core owns (item b, half h): Q/scores/AV/Wo/MLP/LN for its 256 local
   positions, K/V redundantly for the full 512 positions.
 - One 2-core AllGather of fp16 activations per layer boundary.
 - Unembedding position-sharded (full vocab per core), softmax denominator
   via fused activation accumulate.

Precision plan:
 - Q/K projections in fp8e4 DoubleRow matmuls (2x PE rate): their
   quantization noise only jitters softmax scores, which averages out.
 - Everything on the value path (V fill, exp, AV, deno, y, Wo) plus MLP,
   LN and unembed stays fp16 with fp32 PSUM accumulation.
 - Residual adds are injected into the Wo/W2 PSUM by an identity matmul
   (frees the vector engine; PSUM zero-region seeded by even-dt start).
"""
import sys

sys.path.insert(0, '/opt/trn_rl_repo')
import numpy as np

import concourse.bacc as bacc
import concourse.tile as tile
import concourse.mybir as mybir
from concourse.bass_utils import run_bass_kernel_spmd

DT = mybir.dt
F16 = DT.float16
F32 = DT.float32
F8 = DT.float8e4
DR = mybir.MatmulPerfMode.DoubleRow
AF = mybir.ActivationFunctionType
ALU = mybir.AluOpType

N_CORES = 8
P = 128
DE = 1024          # model dim        (8 ptiles)
KO = DE // P       # 8
DMLP = 4096        # mlp dim          (32 ptiles)
MO = DMLP // P     # 32
H = 16             # heads
DA = 64            # attn dim per head
L = 512            # sequence length
LL = 256           # local positions per core
NV = 32000
UC = 500           # unembed vocab chunk (moving N)
UNC = NV // UC     # 64 chunks
LENC = 2
LDEC = 2
EPS = 1e-5

WS = 32.0                     # fp8 weight scale for Wq/Wk
EXPS = 1.0 / (WS * WS * 8.0)  # exp scale (folds q8*k8 scale + sqrt(da))

PAIR_GROUPS = [[0, 1], [2, 3], [4, 5], [6, 7]]

_CACHE = {}


# ----------------------------------------------------------------------------
# device program
# ----------------------------------------------------------------------------

def _attn(nc, tc, pools, pres, e16res, qin8, kv8, kv16, wq_t, wk_t,
          wv_dram, wo_dram, mask16, qs, name):
    """One multi-head attention block. Leaves pres [128, 8, 256] f32 psum
    holding attn_out + residual.

    qin8  : [128, KO, LL] fp8  local stream (query input)
    kv8   : [128, KO, L]  fp8  full-seq stream (key input)
    kv16  : [128, KO, L]  fp16 full-seq stream (value input)
    wq_t/wk_t : SBUF fp8 weight tiles [128, KO, 1024] (x32 scaled)
    wv_dram/wo_dram : fp16 dram APs [128, KO, 1024]
    mask16: [128, 4, LL] fp16 sbuf tile or None
    """
    sb = pools['att']
    pA = pools['pA']
    KT = L // P  # 4 kz tiles

    # ---- residual identity injection (start=True on even dt pre-zeroes
    # the 2KB psum zero-region covering the odd partner as well).
    for dt in range(KO):
        nc.tensor.matmul(pres[:, dt, :], pools['diag1'][:, :],
                         e16res[:, dt, :], start=(dt % 2 == 0), stop=False,
                         skip_group_check=True)

    # prefetch V weights (used after Q/K fills)
    wvts = []
    for nch in range(2):
        wvt = pools['wvp'].tile([P, KO, 512], F16, tag='wvt')
        qs[nch % 2].dma_start(wvt[:], wv_dram[:, :, nch * 512:(nch + 1) * 512])
        wvts.append(wvt)

    # ---- q16: [128(2h x 64a), pr, LL]  (fp8 DR matmul, fp16 result)
    q16 = sb.tile([P, KO, LL], F16, tag='q16')
    for pr in range(KO):
        ps = pA.tile([P, L], F32, tag='pA')
        for j in range(KO // 2):
            nc.tensor.matmul(ps[:, :LL], wq_t[:, 2 * j:2 * j + 2,
                                              pr * P:(pr + 1) * P],
                             qin8[:, 2 * j:2 * j + 2, :],
                             start=(j == 0), stop=(j == 3), perf_mode=DR)
        nc.scalar.activation(q16[:, pr, :], ps[:, :LL], AF.Copy)
    # ---- k16: [128, pr, L]
    k16 = sb.tile([P, KO, L], F16, tag='k16')
    for pr in range(KO):
        ps = pA.tile([P, L], F32, tag='pA')
        for j in range(KO // 2):
            nc.tensor.matmul(ps[:], wk_t[:, 2 * j:2 * j + 2,
                                         pr * P:(pr + 1) * P],
                             kv8[:, 2 * j:2 * j + 2, :],
                             start=(j == 0), stop=(j == 3), perf_mode=DR)
        nc.vector.tensor_copy(k16[:, pr, :], ps[:])
    # ---- vT: [128(kz), kt, 1024(h*64)] fp16
    vt16 = sb.tile([P, KT, H * DA], F16, tag='vt16')
    for nch in range(2):
        wvt = wvts[nch]
        for kt in range(KT):
            ps = pA.tile([P, L], F32, tag='pA')
            for k in range(KO):
                nc.tensor.matmul(ps[:], kv16[:, k, kt * P:(kt + 1) * P],
                                 wvt[:, k, :],
                                 start=(k == 0), stop=(k == KO - 1))
            nc.vector.tensor_copy(vt16[:, kt, nch * 512:(nch + 1) * 512],
                                  ps[:])

    # prefetch Wo chunks (consumed after the head loop)
    wots = []
    for c in range(4):
        wot = pools['wop'].tile([P, KO, 2 * P], F16, tag='wot')
        qs[c % 2].dma_start(wot[:], wo_dram[:, :, c * 2 * P:(c + 1) * 2 * P])
        wots.append(wot)

    # ---- per head pair: scores -> exp -> deno -> AV -> y16
    y16 = sb.tile([P, KO, LL], F16, tag='y16')
    for p in range(KO):
        hA = 2 * p
        # expp [128(kz), kt, hh, q] fp16 for this pair
        expp = sb.tile([P, KT, 2, LL], F16, tag='expp', bufs=2)
        for hh in range(2):
            h = hA + hh
            pr, hp = h // 2, (h % 2) * DA
            for t in range(2):
                ps = pA.tile([P, L], F32, tag='pA')
                for kt in (2 * t, 2 * t + 1):
                    nc.tensor.matmul(
                        ps[:, (kt - 2 * t) * LL:(kt - 2 * t + 1) * LL],
                        k16[hp:hp + DA, pr, kt * P:(kt + 1) * P],
                        q16[hp:hp + DA, pr, :],
                        start=True, stop=True)
                nc.scalar.activation(
                    expp[:, 2 * t:2 * t + 2, hh, :],
                    ps[:].rearrange('p (a b) -> p a b', a=2),
                    AF.Exp, scale=EXPS)
            if mask16 is not None:
                nc.vector.tensor_tensor(
                    expp[:, :, hh, :], expp[:, :, hh, :],
                    mask16[:], ALU.mult)
        # deno: ones-matmul -> [128, (hh, q)]
        pd = pA.tile([P, L], F32, tag='pA')
        for kt in range(KT):
            nc.tensor.matmul(pd[:], pools['ones16'][:, :],
                             expp[:, kt, :, :],
                             start=(kt == 0), stop=(kt == KT - 1))
        ysc = pools['yscp'].tile([P, 2 * LL], F32, tag='ysc')
        nc.vector.reciprocal_approx_fast(ysc[:], pd[:])
        # AV, pair-packed via column tiling
        pav = pA.tile([P, L], F32, tag='pA')
        for hh in range(2):
            hcol = (hA + hh) * DA
            for kt in range(KT):
                nc.tensor.matmul(
                    pav[hh * DA:(hh + 1) * DA, :LL],
                    vt16[:, kt, hcol:hcol + DA],
                    expp[:, kt, hh, :],
                    start=(kt == 0), stop=(kt == KT - 1),
                    tile_position=(0, hh * DA))
        nc.vector.tensor_tensor(y16[:DA, p, :], pav[:DA, :LL],
                                ysc[:DA, 0:LL], ALU.mult)
        nc.vector.tensor_tensor(y16[DA:, p, :], pav[DA:, :LL],
                                ysc[DA:, LL:2 * LL], ALU.mult)

    # ---- Wo accumulate into pres (start=False: id-matmul zeroed/seeded)
    for c in range(4):
        wot = wots[c]
        for di in range(2):
            dt = 2 * c + di
            for k in range(KO):
                nc.tensor.matmul(pres[:, dt, :],
                                 wot[:, k, di * P:(di + 1) * P],
                                 y16[:, k, :],
                                 start=False, stop=(k == KO - 1),
                                 skip_group_check=True)
    tp = pools.get('tapfn')
    if tp:
        tp(f'{name}_q', q16); tp(f'{name}_k', k16); tp(f'{name}_vt', vt16)
        tp(f'{name}_y', y16)


def _mlp(nc, tc, pools, pres, e16res, w1_dram, w2_dram, qs, name):
    """MLP block (fp16). Leaves pres holding mlp_out + residual."""
    pA = pools['pA']
    for dt in range(KO):
        nc.tensor.matmul(pres[:, dt, :], pools['diag1'][:, :],
                         e16res[:, dt, :], start=(dt % 2 == 0), stop=False,
                         skip_group_check=True)
    h16 = pools['mlp'].tile([P, MO, LL], F16, tag='h16')
    for c in range(8):
        w1t = pools['w1p'].tile([P, KO, 4 * P], F16, tag='w1t')
        qs[c % 2].dma_start(w1t[:], w1_dram[:, :, c * 512:(c + 1) * 512])
        for m2 in range(2):   # 2 mt per psum tile
            ps = pA.tile([P, L], F32, tag='pA')
            for mi in range(2):
                for k in range(KO):
                    nc.tensor.matmul(ps[:, mi * LL:(mi + 1) * LL],
                                     w1t[:, k, (2 * m2 + mi) * P:
                                         (2 * m2 + mi + 1) * P],
                                     e16res[:, k, :],
                                     start=(k == 0), stop=(k == KO - 1))
            nc.scalar.activation(h16[:, 4 * c + 2 * m2:4 * c + 2 * m2 + 2, :],
                                 ps[:].rearrange('p (a b) -> p a b', a=2),
                                 AF.Relu)
    for c in range(8):
        w2t = pools['w2p'].tile([P, 4, KO * P], F16, tag='w2t')
        qs[c % 2].dma_start(w2t[:], w2_dram[:, c * 4:(c + 1) * 4, :])
        for j in range(4):
            for dt in range(KO):
                nc.tensor.matmul(pres[:, dt, :],
                                 w2t[:, j, dt * P:(dt + 1) * P],
                                 h16[:, c * 4 + j, :],
                                 start=False,
                                 stop=(c == 7 and j == 3),
                                 skip_group_check=True)


def _ln(nc, tc, pools, pres, e16out, e8out, name):
    """Layernorm from psum pres -> fp16 (+fp8) stream."""
    pA = pools['pA']
    lnp = pools['lnp']
    stat = pools['stat']
    pre16 = lnp.tile([P, KO, LL], F16, tag='pre16')
    nc.scalar.activation(pre16[:], pres[:], AF.Copy)
    sq16 = lnp.tile([P, KO, LL], F16, tag='sq16')
    nc.vector.tensor_tensor(sq16[:], pre16[:], pre16[:], ALU.mult)
    pss = pA.tile([P, L], F32, tag='pA')
    for k in range(KO):
        nc.tensor.matmul(pss[:, :LL], pools['ones16'][:, :], pre16[:, k, :],
                         start=(k == 0), stop=(k == KO - 1))
    psq = pA.tile([P, L], F32, tag='pA')
    for k in range(KO):
        nc.tensor.matmul(psq[:, :LL], pools['ones16'][:, :], sq16[:, k, :],
                         start=(k == 0), stop=(k == KO - 1))
    # v1 = Q - S^2/1024 (= 1023*var); inv = 1/sqrt(v1/1023 + eps);
    # nm = -S/1024 * inv
    s2 = stat.tile([P, LL], F32, tag='s2')
    nc.scalar.activation(s2[:], pss[:, :LL], AF.Square)
    v1 = stat.tile([P, LL], F32, tag='v1')
    nc.vector.scalar_tensor_tensor(v1[:], s2[:], -1.0 / 1024.0, psq[:, :LL],
                                   ALU.mult, ALU.add)
    std = stat.tile([P, LL], F32, tag='std')
    nc.scalar.activation(std[:], v1[:], AF.Sqrt, bias=pools['eps128'],
                         scale=1.0 / 1023.0)
    inv = stat.tile([P, LL], F32, tag='inv')
    nc.vector.reciprocal_approx_fast(inv[:], std[:])
    nm = stat.tile([P, LL], F32, tag='nm')
    nc.vector.scalar_tensor_tensor(nm[:], pss[:, :LL], -1.0 / 1024.0, inv[:],
                                   ALU.mult, ALU.mult)
    nc.vector.tensor_tensor(
        e16out[:], pre16[:],
        inv[:, None, :].to_broadcast((P, KO, LL)), ALU.mult)
    nc.vector.tensor_tensor(
        e16out[:], e16out[:],
        nm[:, None, :].to_broadcast((P, KO, LL)), ALU.add)
    if e8out is not None:
        nc.vector.tensor_copy(e8out[:], e16out[:])
    tp = pools.get('tapfn')
    if tp:
        tp(f'{name}_out', e16out)


def _allgather_pair(nc, tc, pools, e16loc, full16, full8, agin, agout, tag):
    """e16loc [128, KO, LL] -> pair AllGather -> full16/full8 [128, KO, L]."""
    nc.gpsimd.dma_start(agin[:], e16loc[:])
    nc.gpsimd.collective_compute(
        "AllGather", ALU.bypass,
        ins=[agin[:]], outs=[agout[:]],
        replica_groups=PAIR_GROUPS)
    nc.gpsimd.dma_start(
        full16[:].rearrange('ki ko (r p) -> ki ko r p', r=2),
        agout[:].rearrange('r ki ko p -> ki ko r p'))
    nc.vector.tensor_copy(full8[:], full16[:])


def build_program(taps=()):
    taps = set(taps)
    nc = bacc.Bacc("TRN2", target_bir_lowering=False, debug=False,
                   num_devices=N_CORES)

    # ---- dram inputs ----
    din = {}
    def dram_in(nm, shape, dt=F16):
        din[nm] = nc.dram_tensor(nm, list(shape), dt, kind="ExternalInput")
        return din[nm]

    z0f16 = dram_in('z0_full16', [P, KO, L])
    x0f16 = dram_in('x0_full16', [P, KO, L])
    z0f8 = dram_in('z0_full8', [P, KO, L], F8)
    x0f8 = dram_in('x0_full8', [P, KO, L], F8)
    z0l8 = dram_in('z0_loc8', [P, KO, LL], F8)
    x0l8 = dram_in('x0_loc8', [P, KO, LL], F8)
    z0l16 = dram_in('z0_loc16', [P, KO, LL])
    x0l16 = dram_in('x0_loc16', [P, KO, LL])
    mask_self = dram_in('mask_self16', [P, 4, LL])
    diag1_d = dram_in('diag1', [P, P])
    for pfx, nl in (('enc', LENC), ('dec', LDEC)):
        for w in ('wq8', 'wk8'):
            dram_in(f'{pfx}_{w}', [nl, P, KO, DE], F8)
        for w in ('wvT', 'woT'):
            dram_in(f'{pfx}_{w}', [nl, P, KO, DE])
        dram_in(f'{pfx}_w1T', [nl, P, KO, DMLP])
        dram_in(f'{pfx}_w2T', [nl, P, MO, DE])
    wuT = dram_in('wuT', [P, KO, NV])

    # output: [pt, ki, vocab] fp16 UNNORMALIZED exp(logits),
    # positions = h*256 + pt*128 + ki; deno_d = softmax denominators.
    outp = nc.dram_tensor('outp', [2, P, NV], F16, kind="ExternalOutput")
    deno_d = nc.dram_tensor('deno_out', [P, 2], F32, kind="ExternalOutput")

    # internal dram for pair collectives
    agin = nc.dram_tensor('agin', [P, KO, LL], F16)
    agout = nc.dram_tensor('agout', [2, P, KO, LL], F16)
    wuin = nc.dram_tensor('wuin', [1, 64], F16)
    wuout = nc.dram_tensor('wuout', [2, 64], F16)

    import contextlib
    with tile.TileContext(nc) as tc, contextlib.ExitStack() as octx:
        const = octx.enter_context(tc.tile_pool(name='const', bufs=1))
        ones16 = const.tile([P, P], F16)
        nc.vector.memset(ones16[:], 1.0)
        eps128 = const.tile([P, 1], F32)
        nc.vector.memset(eps128[:], EPS)
        diag1 = const.tile([P, P], F16)
        nc.sync.dma_start(diag1[:], diag1_d[:])
        msk16 = const.tile([P, 4, LL], F16)
        nc.sync.dma_start(msk16[:], mask_self[:])
        xu = const.tile([P, KO, LL], F16)
        # warmup collective: absorbs the cc-infrastructure spin-up (~40us)
        # off the critical path before the first real AllGather.
        wut0 = const.tile([1, 64], F16)
        nc.vector.memset(wut0[:], 0.0)
        nc.gpsimd.dma_start(wuin[:], wut0[:])
        nc.gpsimd.collective_compute(
            "AllGather", ALU.bypass, ins=[wuin[:]], outs=[wuout[:]],
            replica_groups=PAIR_GROUPS)

        # ================= layer phase =================
        with contextlib.ExitStack() as ctx:
            stream = ctx.enter_context(tc.tile_pool(name='stream', bufs=1))
            att = ctx.enter_context(tc.tile_pool(name='att', bufs=1))
            mlpp = ctx.enter_context(tc.tile_pool(name='mlpp', bufs=1))
            lnp = ctx.enter_context(tc.tile_pool(name='lnp', bufs=1))
            stat = ctx.enter_context(tc.tile_pool(name='stat', bufs=1))
            yscp = ctx.enter_context(tc.tile_pool(name='yscp', bufs=2))
            watp = ctx.enter_context(tc.tile_pool(name='watp', bufs=1))
            wvp = ctx.enter_context(tc.tile_pool(name='wvp', bufs=2))
            wop = ctx.enter_context(tc.tile_pool(name='wop', bufs=4))
            w1p = ctx.enter_context(tc.tile_pool(name='w1p', bufs=2))
            w2p = ctx.enter_context(tc.tile_pool(name='w2p', bufs=2))
            pA = ctx.enter_context(tc.tile_pool(name='pA', bufs=4,
                                                space='PSUM'))
            presp = ctx.enter_context(tc.tile_pool(name='presp', bufs=1,
                                                   space='PSUM'))

            pools = dict(att=att, mlp=mlpp, lnp=lnp, stat=stat, yscp=yscp,
                         pA=pA, wvp=wvp, wop=wop, w1p=w1p, w2p=w2p,
                         ones16=ones16, eps128=eps128[:], diag1=diag1)

            def tapfn(nm, t):
                if nm not in taps:
                    return
                d = nc.dram_tensor('tap_' + nm, list(t.shape),
                                   t.dtype, kind="ExternalOutput")
                nc.sync.dma_start(d[:], t[:])
            pools['tapfn'] = tapfn

            qs = [nc.sync, nc.scalar]

            def load_qk(pfx, l):
                wq = watp.tile([P, KO, DE], F8, tag='w_wq8')
                qs[0].dma_start(wq[:], din[f'{pfx}_wq8'][l])
                wk = watp.tile([P, KO, DE], F8, tag='w_wk8')
                qs[1].dma_start(wk[:], din[f'{pfx}_wk8'][l])
                return wq, wk

            # ======== encoder ========
            e16 = stream.tile([P, KO, LL], F16, tag='e16_a')
            nc.sync.dma_start(e16[:], z0l16[:])
            e8 = stream.tile([P, KO, LL], F8, tag='e8_a')
            nc.scalar.dma_start(e8[:], z0l8[:])
            Zfull = stream.tile([P, KO, L], F16, tag='Zfull')
            nc.sync.dma_start(Zfull[:], z0f16[:])
            Zfull8 = stream.tile([P, KO, L], F8, tag='Zfull8')
            nc.scalar.dma_start(Zfull8[:], z0f8[:])

            pres = presp.tile([P, KO, LL], F32, tag='pres')
            for l in range(LENC):
                wq, wk = load_qk('enc', l)
                _attn(nc, tc, pools, pres, e16, e8, Zfull8, Zfull,
                      wq, wk, din['enc_wvT'][l], din['enc_woT'][l],
                      None, qs, f'e{l}a')
                e16 = stream.tile([P, KO, LL], F16, tag='e16_b')
                e8 = stream.tile([P, KO, LL], F8, tag='e8_b')
                _ln(nc, tc, pools, pres, e16, e8, f'e{l}ln1')
                _mlp(nc, tc, pools, pres, e16, din['enc_w1T'][l],
                     din['enc_w2T'][l], qs, f'e{l}m')
                e16 = stream.tile([P, KO, LL], F16, tag='e16_a')
                e8 = stream.tile([P, KO, LL], F8, tag='e8_a')
                _ln(nc, tc, pools, pres, e16, e8, f'e{l}ln2')
                Zfull = stream.tile([P, KO, L], F16, tag='Zfull')
                Zfull8 = stream.tile([P, KO, L], F8, tag='Zfull8')
                _allgather_pair(nc, tc, pools, e16, Zfull, Zfull8,
                                agin, agout, f'e{l}')

            # ======== decoder ========
            e16 = stream.tile([P, KO, LL], F16, tag='e16_b')
            nc.sync.dma_start(e16[:], x0l16[:])
            e8 = stream.tile([P, KO, LL], F8, tag='e8_b')
            nc.scalar.dma_start(e8[:], x0l8[:])
            Xfull = stream.tile([P, KO, L], F16, tag='Xfull')
            nc.sync.dma_start(Xfull[:], x0f16[:])
            Xfull8 = stream.tile([P, KO, L], F8, tag='Xfull8')
            nc.scalar.dma_start(Xfull8[:], x0f8[:])

            for l in range(LDEC):
                wq, wk = load_qk('dec', l)
                _attn(nc, tc, pools, pres, e16, e8, Xfull8, Xfull,
                      wq, wk, din['dec_wvT'][l], din['dec_woT'][l],
                      msk16, qs, f'd{l}s')
                e16 = stream.tile([P, KO, LL], F16, tag='e16_a')
                e8 = stream.tile([P, KO, LL], F8, tag='e8_a')
                _ln(nc, tc, pools, pres, e16, e8, f'd{l}ln1')
                _attn(nc, tc, pools, pres, e16, e8, Zfull8, Zfull,
                      wq, wk, din['dec_wvT'][l], din['dec_woT'][l],
                      None, qs, f'd{l}c')
                e16 = stream.tile([P, KO, LL], F16, tag='e16_b')
                e8 = stream.tile([P, KO, LL], F8, tag='e8_b')
                _ln(nc, tc, pools, pres, e16, e8, f'd{l}ln2')
                _mlp(nc, tc, pools, pres, e16, din['dec_w1T'][l],
                     din['dec_w2T'][l], qs, f'd{l}m')
                if l < LDEC - 1:
                    e16 = stream.tile([P, KO, LL], F16, tag='e16_a')
                    e8 = stream.tile([P, KO, LL], F8, tag='e8_a')
                    _ln(nc, tc, pools, pres, e16, e8, f'd{l}ln3')
                    Xfull = stream.tile([P, KO, L], F16, tag='Xfull')
                    Xfull8 = stream.tile([P, KO, L], F8, tag='Xfull8')
                    _allgather_pair(nc, tc, pools, e16, Xfull, Xfull8,
                                    agin, agout, f'd{l}')
                else:
                    _ln(nc, tc, pools, pres, xu, None, f'd{l}ln3')

        # ========== unembed phase (position-sharded, no collectives) ========
        # Emits UNNORMALIZED exp(logits) per chunk (DMA'd out immediately,
        # overlapping the remaining matmuls) plus the per-position softmax
        # denominator; the host divides during assembly.
        with contextlib.ExitStack() as ctx:
            usb = ctx.enter_context(tc.tile_pool(name='usb', bufs=1))
            wup = ctx.enter_context(tc.tile_pool(name='wup', bufs=4))
            expp_ = ctx.enter_context(tc.tile_pool(name='expp_', bufs=4))
            up = ctx.enter_context(tc.tile_pool(name='up', bufs=6,
                                                space='PSUM'))
            dacc = usb.tile([P, 2, UNC], F32, tag='dacc')
            # dedicated queues: wut loads on scalar's hwdge, outputs on
            # sync's — avoids head-of-line blocking between them.
            for c in range(UNC):
                wut = wup.tile([P, KO, UC], F16, tag='wut')
                nc.scalar.dma_start(wut[:], wuT[:, :, c * UC:(c + 1) * UC])
                for pt in range(2):
                    ps = up.tile([P, UC], F32, tag='ups', name=f'ups{c}{pt}')
                    for k in range(KO):
                        nc.tensor.matmul(ps[:], xu[:, k, pt * P:(pt + 1) * P],
                                         wut[:, k, :], start=(k == 0),
                                         stop=(k == KO - 1))
                    ex = expp_.tile([P, UC], F16, tag='uex')
                    nc.scalar.activation(ex[:], ps[:], AF.Exp,
                                         accum_out=dacc[:, pt, c:c + 1])
                    nc.sync.dma_start(
                        outp[pt, :, c * UC:(c + 1) * UC], ex[:])
            deno = usb.tile([P, 2], F32, tag='deno')
            nc.vector.tensor_reduce(deno[:], dacc[:], mybir.AxisListType.X,
                                    ALU.add)
            nc.sync.dma_start(deno_d[:], deno[:])

    nc.compile()
    return nc


# ----------------------------------------------------------------------------
# host-side prep
# ----------------------------------------------------------------------------

def _to_kimaj(a):
    """[K, M] -> [128, K//128, M] with K = ko*128 + ki."""
    K, M = a.shape
    return np.ascontiguousarray(
        a.reshape(K // P, P, M).transpose(1, 0, 2))


def _fp8(a):
    return np.clip(a, -240.0, 240.0).astype(mybir.dt.np(F8))


def prep_inputs(inputs):
    f = lambda k: np.asarray(inputs[k], dtype=np.float32)
    We, Wp, Wu = f('We'), f('Wp'), f('Wu')
    x = np.asarray(inputs['x']).astype(np.int64)
    z = np.asarray(inputs['z']).astype(np.int64)

    shared = {}
    for pfx, nl in (('enc', LENC), ('dec', LDEC)):
        Wq, Wk, Wv = f(pfx + '_Wq'), f(pfx + '_Wk'), f(pfx + '_Wv')
        Wo, W1, W2 = f(pfx + '_Wo'), f(pfx + '_W1'), f(pfx + '_W2')
        wq, wk, wv, wo, w1, w2 = [], [], [], [], [], []
        for l in range(nl):
            qa = Wq[l].reshape(H * DA, DE).T * WS
            ka = Wk[l].reshape(H * DA, DE).T * WS
            va = Wv[l].transpose(2, 0, 1).reshape(DE, H * DA)
            wq.append(_to_kimaj(qa)); wk.append(_to_kimaj(ka))
            wv.append(_to_kimaj(va))
            wo.append(_to_kimaj(Wo[l].T))
            w1.append(_to_kimaj(W1[l].T))
            w2.append(_to_kimaj(W2[l].T))
        shared[f'{pfx}_wq8'] = _fp8(np.stack(wq))
        shared[f'{pfx}_wk8'] = _fp8(np.stack(wk))
        shared[f'{pfx}_wvT'] = np.stack(wv).astype(np.float16)
        shared[f'{pfx}_woT'] = np.stack(wo).astype(np.float16)
        shared[f'{pfx}_w1T'] = np.stack(w1).astype(np.float16)
        shared[f'{pfx}_w2T'] = np.stack(w2).astype(np.float16)

    shared['wuT'] = _to_kimaj(Wu.T).astype(np.float16)
    shared['diag1'] = np.eye(P, dtype=np.float16)

    pos = Wp[:L]  # [512, 1024]
    in_maps = []
    for c in range(N_CORES):
        b, h = c // 2, c % 2
        m = dict(shared)
        for nm, tok in (('z0', z[b]), ('x0', x[b])):
            E0 = (We[tok] + pos).T.astype(np.float32)      # [1024, 512]
            E0k = E0.reshape(KO, P, L)                     # [ko, ki, p]
            full = np.ascontiguousarray(E0k.transpose(1, 0, 2))
            m[nm + '_full16'] = full.astype(np.float16)
            m[nm + '_full8'] = _fp8(full)
            loc = np.ascontiguousarray(
                E0k[:, :, h * LL:(h + 1) * LL].transpose(1, 0, 2))
            m[nm + '_loc16'] = loc.astype(np.float16)
            m[nm + '_loc8'] = _fp8(loc)
        kglob = np.arange(L)[:, None]
        qglob = (h * LL + np.arange(LL))[None, :]
        msk = (kglob <= qglob).astype(np.float16)          # [512, 256]
        m['mask_self16'] = np.ascontiguousarray(
            msk.reshape(4, P, LL).transpose(1, 0, 2))
        in_maps.append(m)
    return in_maps


def assemble(results):
    """results: per-core dicts with 'outp' [2, 128, NV] fp16 (unnormalized
    exp of logits) and 'deno_out' [128, 2] f32 softmax denominators."""
    out = np.empty((4, NV, L), dtype=np.float32)
    for c, r in enumerate(results):
        b, h = c // 2, c % 2
        o = r['outp'].astype(np.float32).reshape(LL, NV)   # [pos, vocab]
        deno = r['deno_out'].T.reshape(LL, 1)              # [pos, 1]
        out[b, :, h * LL:(h + 1) * LL] = (o / deno).T
    return out


def run(inputs, trace=False, taps=(), trace_kwargs=None):
    key = ('prog', tuple(sorted(taps)))
    if key not in _CACHE:
        _CACHE[key] = build_program(taps=taps)
    nc = _CACHE[key]
    in_maps = prep_inputs(inputs)
    res = run_bass_kernel_spmd(nc, in_maps, list(range(N_CORES)),
                               trace=trace, **(trace_kwargs or {}))
    return res


def kernel(**inputs):
    res = run(inputs, trace=False)
    return assemble(res.results)


# revision 27
# speedup vs baseline: 1.3014x; 1.2962x over previous
"""Trainium2 Bass kernel for nn_EDTransformer (encoder-decoder transformer).

Sharding: 8 cores = 4 batch items x 2 sequence halves.
 - Each core owns (item b, half h): Q/scores/AV/Wo/MLP/LN for its 256 local
   positions, K/V redundantly for the full 512 positions.
 - One 2-core AllGather of fp16 activations per layer boundary.
 - Unembedding position-sharded (full vocab per core), softmax denominator
   via fused activation accumulate.

Precision plan:
 - Q/K projections in fp8e4 DoubleRow matmuls (2x PE rate): their
   quantization noise only jitters softmax scores, which averages out.
 - Everything on the value path (V fill, exp, AV, deno, y, Wo) plus MLP,
   LN and unembed stays fp16 with fp32 PSUM accumulation.
 - Residual adds are injected into the Wo/W2 PSUM by an identity matmul
   (frees the vector engine; PSUM zero-region seeded by even-dt start).
"""
import sys

sys.path.insert(0, '/opt/trn_rl_repo')
import numpy as np

import concourse.bacc as bacc
import concourse.tile as tile
import concourse.mybir as mybir
from concourse.bass_utils import run_bass_kernel_spmd

DT = mybir.dt
F16 = DT.float16
F32 = DT.float32
F8 = DT.float8e4
DR = mybir.MatmulPerfMode.DoubleRow
AF = mybir.ActivationFunctionType
ALU = mybir.AluOpType

N_CORES = 8
P = 128
DE = 1024          # model dim        (8 ptiles)
KO = DE // P       # 8
DMLP = 4096        # mlp dim          (32 ptiles)
MO = DMLP // P     # 32
H = 16             # heads
DA = 64            # attn dim per head
L = 512            # sequence length
LL = 256           # local positions per core
NV = 32000
UC = 500           # unembed vocab chunk (moving N)
UNC = NV // UC     # 64 chunks
LENC = 2
LDEC = 2
EPS = 1e-5

WS = 32.0                     # fp8 weight scale for Wq/Wk
EXPS = 1.0 / (WS * WS * 8.0)  # exp scale (folds q8*k8 scale + sqrt(da))

PAIR_GROUPS = [[0, 1], [2, 3], [4, 5], [6, 7]]

_CACHE = {}


# ----------------------------------------------------------------------------
# device program
# ----------------------------------------------------------------------------

def _attn(nc, tc, pools, pres, e16res, qin8, kv8, kv16, wq_t, wk_t,
          wv_dram, wo_dram, mask16, qs, name):
    """One multi-head attention block. Leaves pres [128, 8, 256] f32 psum
    holding attn_out + residual.

    qin8  : [128, KO, LL] fp8  local stream (query input)
    kv8   : [128, KO, L]  fp8  full-seq stream (key input)
    kv16  : [128, KO, L]  fp16 full-seq stream (value input)
    wq_t/wk_t : SBUF fp8 weight tiles [128, KO, 1024] (x32 scaled)
    wv_dram/wo_dram : fp16 dram APs [128, KO, 1024]
    mask16: [128, 4, LL] fp16 sbuf tile or None
    """
    sb = pools['att']
    pA = pools['pA']
    KT = L // P  # 4 kz tiles

    # ---- residual identity injection (start=True on even dt pre-zeroes
    # the 2KB psum zero-region covering the odd partner as well).
    for dt in range(KO):
        nc.tensor.matmul(pres[:, dt, :], pools['diag1'][:, :],
                         e16res[:, dt, :], start=(dt % 2 == 0), stop=False,
                         skip_group_check=True)

    # prefetch V weights (used after Q/K fills)
    wvts = []
    for nch in range(2):
        wvt = pools['wvp'].tile([P, KO, 512], F16, tag='wvt')
        qs[nch % 2].dma_start(wvt[:], wv_dram[:, :, nch * 512:(nch + 1) * 512])
        wvts.append(wvt)

    # ---- q16: [128(2h x 64a), pr, LL]  (fp8 DR matmul, fp16 result)
    q16 = sb.tile([P, KO, LL], F16, tag='q16')
    for pr in range(KO):
        ps = pA.tile([P, L], F32, tag='pA')
        for j in range(KO // 2):
            nc.tensor.matmul(ps[:, :LL], wq_t[:, 2 * j:2 * j + 2,
                                              pr * P:(pr + 1) * P],
                             qin8[:, 2 * j:2 * j + 2, :],
                             start=(j == 0), stop=(j == 3), perf_mode=DR)
        nc.scalar.activation(q16[:, pr, :], ps[:, :LL], AF.Copy)
    # ---- k16: [128, pr, L]
    k16 = sb.tile([P, KO, L], F16, tag='k16')
    for pr in range(KO):
        ps = pA.tile([P, L], F32, tag='pA')
        for j in range(KO // 2):
            nc.tensor.matmul(ps[:], wk_t[:, 2 * j:2 * j + 2,
                                         pr * P:(pr + 1) * P],
                             kv8[:, 2 * j:2 * j + 2, :],
                             start=(j == 0), stop=(j == 3), perf_mode=DR)
        nc.vector.tensor_copy(k16[:, pr, :], ps[:])
    # ---- vT: [128(kz), kt, 1024(h*64)] fp16
    vt16 = sb.tile([P, KT, H * DA], F16, tag='vt16')
    for nch in range(2):
        wvt = wvts[nch]
        for kt in range(KT):
            ps = pA.tile([P, L], F32, tag='pA')
            for k in range(KO):
                nc.tensor.matmul(ps[:], kv16[:, k, kt * P:(kt + 1) * P],
                                 wvt[:, k, :],
                                 start=(k == 0), stop=(k == KO - 1))
            nc.vector.tensor_copy(vt16[:, kt, nch * 512:(nch + 1) * 512],
                                  ps[:])

    # prefetch Wo chunks (consumed after the head loop)
    wots = []
    for c in range(4):
        wot = pools['wop'].tile([P, KO, 2 * P], F16, tag='wot')
        qs[c % 2].dma_start(wot[:], wo_dram[:, :, c * 2 * P:(c + 1) * 2 * P])
        wots.append(wot)

    # ---- per head pair: scores -> exp -> deno -> AV -> y16.
    # Software-pipelined by one pair: scores/exp for pair p are issued
    # before deno/AV of pair p-1 so the tensor engine (in-order) never
    # blocks on the scalar exp of the pair it is about to reduce.
    y16 = sb.tile([P, KO, LL], F16, tag='y16')
    expps = {}

    def emit_scores(p):
        hA = 2 * p
        expp = sb.tile([P, KT, 2, LL], F16, tag='expp', bufs=2)
        expps[p] = expp
        for hh in range(2):
            h = hA + hh
            pr, hp = h // 2, (h % 2) * DA
            for t in range(2):
                ps = pA.tile([P, L], F32, tag='pA')
                for kt in (2 * t, 2 * t + 1):
                    nc.tensor.matmul(
                        ps[:, (kt - 2 * t) * LL:(kt - 2 * t + 1) * LL],
                        k16[hp:hp + DA, pr, kt * P:(kt + 1) * P],
                        q16[hp:hp + DA, pr, :],
                        start=True, stop=True)
                nc.scalar.activation(
                    expp[:, 2 * t:2 * t + 2, hh, :],
                    ps[:].rearrange('p (a b) -> p a b', a=2),
                    AF.Exp, scale=EXPS)
            if mask16 is not None:
                nc.vector.tensor_tensor(
                    expp[:, :, hh, :], expp[:, :, hh, :],
                    mask16[:], ALU.mult)

    def emit_reduce(p):
        hA = 2 * p
        expp = expps.pop(p)
        # deno: ones-matmul -> [128, (hh, q)]
        pd = pA.tile([P, L], F32, tag='pA')
        for kt in range(KT):
            nc.tensor.matmul(pd[:], pools['ones16'][:, :],
                             expp[:, kt, :, :],
                             start=(kt == 0), stop=(kt == KT - 1))
        ysc = pools['yscp'].tile([P, 2 * LL], F32, tag='ysc')
        nc.vector.reciprocal_approx_fast(ysc[:], pd[:])
        # AV, pair-packed via column tiling
        pav = pA.tile([P, L], F32, tag='pA')
        for hh in range(2):
            hcol = (hA + hh) * DA
            for kt in range(KT):
                nc.tensor.matmul(
                    pav[hh * DA:(hh + 1) * DA, :LL],
                    vt16[:, kt, hcol:hcol + DA],
                    expp[:, kt, hh, :],
                    start=(kt == 0), stop=(kt == KT - 1),
                    tile_position=(0, hh * DA))
        nc.vector.tensor_tensor(y16[:DA, p, :], pav[:DA, :LL],
                                ysc[:DA, 0:LL], ALU.mult)
        nc.vector.tensor_tensor(y16[DA:, p, :], pav[DA:, :LL],
                                ysc[DA:, LL:2 * LL], ALU.mult)

    emit_scores(0)
    for p in range(1, KO):
        emit_scores(p)
        emit_reduce(p - 1)
    emit_reduce(KO - 1)

    # ---- Wo accumulate into pres (start=False: id-matmul zeroed/seeded)
    for c in range(4):
        wot = wots[c]
        for di in range(2):
            dt = 2 * c + di
            for k in range(KO):
                nc.tensor.matmul(pres[:, dt, :],
                                 wot[:, k, di * P:(di + 1) * P],
                                 y16[:, k, :],
                                 start=False, stop=(k == KO - 1),
                                 skip_group_check=True)
    tp = pools.get('tapfn')
    if tp:
        tp(f'{name}_q', q16); tp(f'{name}_k', k16); tp(f'{name}_vt', vt16)
        tp(f'{name}_y', y16)


def _mlp(nc, tc, pools, pres, e16res, w1_dram, w2_dram, qs, name):
    """MLP block (fp16). Leaves pres holding mlp_out + residual."""
    pA = pools['pA']
    for dt in range(KO):
        nc.tensor.matmul(pres[:, dt, :], pools['diag1'][:, :],
                         e16res[:, dt, :], start=(dt % 2 == 0), stop=False,
                         skip_group_check=True)
    h16 = pools['mlp'].tile([P, MO, LL], F16, tag='h16')
    for c in range(8):
        w1t = pools['w1p'].tile([P, KO, 4 * P], F16, tag='w1t')
        qs[c % 2].dma_start(w1t[:], w1_dram[:, :, c * 512:(c + 1) * 512])
        for m2 in range(2):   # 2 mt per psum tile
            ps = pA.tile([P, L], F32, tag='pA')
            for mi in range(2):
                for k in range(KO):
                    nc.tensor.matmul(ps[:, mi * LL:(mi + 1) * LL],
                                     w1t[:, k, (2 * m2 + mi) * P:
                                         (2 * m2 + mi + 1) * P],
                                     e16res[:, k, :],
                                     start=(k == 0), stop=(k == KO - 1))
            nc.scalar.activation(h16[:, 4 * c + 2 * m2:4 * c + 2 * m2 + 2, :],
                                 ps[:].rearrange('p (a b) -> p a b', a=2),
                                 AF.Relu)
    for c in range(8):
        w2t = pools['w2p'].tile([P, 4, KO * P], F16, tag='w2t')
        qs[c % 2].dma_start(w2t[:], w2_dram[:, c * 4:(c + 1) * 4, :])
        for j in range(4):
            for dt in range(KO):
                nc.tensor.matmul(pres[:, dt, :],
                                 w2t[:, j, dt * P:(dt + 1) * P],
                                 h16[:, c * 4 + j, :],
                                 start=False,
                                 stop=(c == 7 and j == 3),
                                 skip_group_check=True)


def _ln(nc, tc, pools, pres, e16out, e8out, name):
    """Layernorm from psum pres -> fp16 (+fp8) stream."""
    pA = pools['pA']
    lnp = pools['lnp']
    stat = pools['stat']
    pre16 = lnp.tile([P, KO, LL], F16, tag='pre16')
    nc.scalar.activation(pre16[:], pres[:], AF.Copy)
    sq16 = lnp.tile([P, KO, LL], F16, tag='sq16')
    nc.vector.tensor_tensor(sq16[:], pre16[:], pre16[:], ALU.mult)
    pss = pA.tile([P, L], F32, tag='pA')
    for k in range(KO):
        nc.tensor.matmul(pss[:, :LL], pools['ones16'][:, :], pre16[:, k, :],
                         start=(k == 0), stop=(k == KO - 1))
    psq = pA.tile([P, L], F32, tag='pA')
    for k in range(KO):
        nc.tensor.matmul(psq[:, :LL], pools['ones16'][:, :], sq16[:, k, :],
                         start=(k == 0), stop=(k == KO - 1))
    # v1 = Q - S^2/1024 (= 1023*var); inv = 1/sqrt(v1/1023 + eps);
    # nm = -S/1024 * inv
    s2 = stat.tile([P, LL], F32, tag='s2')
    nc.scalar.activation(s2[:], pss[:, :LL], AF.Square)
    v1 = stat.tile([P, LL], F32, tag='v1')
    nc.vector.scalar_tensor_tensor(v1[:], s2[:], -1.0 / 1024.0, psq[:, :LL],
                                   ALU.mult, ALU.add)
    std = stat.tile([P, LL], F32, tag='std')
    nc.scalar.activation(std[:], v1[:], AF.Sqrt, bias=pools['eps128'],
                         scale=1.0 / 1023.0)
    inv = stat.tile([P, LL], F32, tag='inv')
    nc.vector.reciprocal_approx_fast(inv[:], std[:])
    nm = stat.tile([P, LL], F32, tag='nm')
    nc.vector.scalar_tensor_tensor(nm[:], pss[:, :LL], -1.0 / 1024.0, inv[:],
                                   ALU.mult, ALU.mult)
    nc.vector.tensor_tensor(
        e16out[:], pre16[:],
        inv[:, None, :].to_broadcast((P, KO, LL)), ALU.mult)
    nc.vector.tensor_tensor(
        e16out[:], e16out[:],
        nm[:, None, :].to_broadcast((P, KO, LL)), ALU.add)
    if e8out is not None:
        nc.vector.tensor_copy(e8out[:], e16out[:])
    tp = pools.get('tapfn')
    if tp:
        tp(f'{name}_out', e16out)


def _allgather_pair(nc, tc, pools, e16loc, full16, full8, agin, agout, tag):
    """e16loc [128, KO, LL] -> pair AllGather -> full16/full8 [128, KO, L]."""
    nc.gpsimd.dma_start(agin[:], e16loc[:])
    nc.gpsimd.collective_compute(
        "AllGather", ALU.bypass,
        ins=[agin[:]], outs=[agout[:]],
        replica_groups=PAIR_GROUPS)
    nc.gpsimd.dma_start(
        full16[:].rearrange('ki ko (r p) -> ki ko r p', r=2),
        agout[:].rearrange('r ki ko p -> ki ko r p'))
    nc.vector.tensor_copy(full8[:], full16[:])


def build_program(taps=()):
    taps = set(taps)
    nc = bacc.Bacc("TRN2", target_bir_lowering=False, debug=False,
                   num_devices=N_CORES)

    # ---- dram inputs ----
    din = {}
    def dram_in(nm, shape, dt=F16):
        din[nm] = nc.dram_tensor(nm, list(shape), dt, kind="ExternalInput")
        return din[nm]

    z0f16 = dram_in('z0_full16', [P, KO, L])
    x0f16 = dram_in('x0_full16', [P, KO, L])
    z0f8 = dram_in('z0_full8', [P, KO, L], F8)
    x0f8 = dram_in('x0_full8', [P, KO, L], F8)
    z0l8 = dram_in('z0_loc8', [P, KO, LL], F8)
    x0l8 = dram_in('x0_loc8', [P, KO, LL], F8)
    z0l16 = dram_in('z0_loc16', [P, KO, LL])
    x0l16 = dram_in('x0_loc16', [P, KO, LL])
    mask_self = dram_in('mask_self16', [P, 4, LL])
    diag1_d = dram_in('diag1', [P, P])
    for pfx, nl in (('enc', LENC), ('dec', LDEC)):
        for w in ('wq8', 'wk8'):
            dram_in(f'{pfx}_{w}', [nl, P, KO, DE], F8)
        for w in ('wvT', 'woT'):
            dram_in(f'{pfx}_{w}', [nl, P, KO, DE])
        dram_in(f'{pfx}_w1T', [nl, P, KO, DMLP])
        dram_in(f'{pfx}_w2T', [nl, P, MO, DE])
    wuT = dram_in('wuT', [P, KO, NV])

    # output: [pt, ki, vocab] fp16 UNNORMALIZED exp(logits),
    # positions = h*256 + pt*128 + ki; deno_d = softmax denominators.
    outp = nc.dram_tensor('outp', [2, P, NV], F16, kind="ExternalOutput")
    deno_d = nc.dram_tensor('deno_out', [P, 2], F32, kind="ExternalOutput")

    # internal dram for pair collectives
    agin = nc.dram_tensor('agin', [P, KO, LL], F16)
    agout = nc.dram_tensor('agout', [2, P, KO, LL], F16)
    wuin = nc.dram_tensor('wuin', [1, 64], F16)
    wuout = nc.dram_tensor('wuout', [2, 64], F16)

    import contextlib
    with tile.TileContext(nc) as tc, contextlib.ExitStack() as octx:
        const = octx.enter_context(tc.tile_pool(name='const', bufs=1))
        ones16 = const.tile([P, P], F16)
        nc.vector.memset(ones16[:], 1.0)
        eps128 = const.tile([P, 1], F32)
        nc.vector.memset(eps128[:], EPS)
        diag1 = const.tile([P, P], F16)
        nc.sync.dma_start(diag1[:], diag1_d[:])
        msk16 = const.tile([P, 4, LL], F16)
        nc.sync.dma_start(msk16[:], mask_self[:])
        xu = const.tile([P, KO, LL], F16)
        # warmup collective: absorbs the cc-infrastructure spin-up (~40us)
        # off the critical path before the first real AllGather.
        wut0 = const.tile([1, 64], F16)
        nc.vector.memset(wut0[:], 0.0)
        nc.gpsimd.dma_start(wuin[:], wut0[:])
        nc.gpsimd.collective_compute(
            "AllGather", ALU.bypass, ins=[wuin[:]], outs=[wuout[:]],
            replica_groups=PAIR_GROUPS)

        # ================= layer phase =================
        with contextlib.ExitStack() as ctx:
            stream = ctx.enter_context(tc.tile_pool(name='stream', bufs=1))
            att = ctx.enter_context(tc.tile_pool(name='att', bufs=1))
            mlpp = ctx.enter_context(tc.tile_pool(name='mlpp', bufs=1))
            lnp = ctx.enter_context(tc.tile_pool(name='lnp', bufs=1))
            stat = ctx.enter_context(tc.tile_pool(name='stat', bufs=1))
            yscp = ctx.enter_context(tc.tile_pool(name='yscp', bufs=2))
            watp = ctx.enter_context(tc.tile_pool(name='watp', bufs=1))
            wvp = ctx.enter_context(tc.tile_pool(name='wvp', bufs=2))
            wop = ctx.enter_context(tc.tile_pool(name='wop', bufs=4))
            w1p = ctx.enter_context(tc.tile_pool(name='w1p', bufs=3))
            w2p = ctx.enter_context(tc.tile_pool(name='w2p', bufs=3))
            pA = ctx.enter_context(tc.tile_pool(name='pA', bufs=4,
                                                space='PSUM'))
            presp = ctx.enter_context(tc.tile_pool(name='presp', bufs=1,
                                                   space='PSUM'))

            pools = dict(att=att, mlp=mlpp, lnp=lnp, stat=stat, yscp=yscp,
                         pA=pA, wvp=wvp, wop=wop, w1p=w1p, w2p=w2p,
                         ones16=ones16, eps128=eps128[:], diag1=diag1)

            def tapfn(nm, t):
                if nm not in taps:
                    return
                d = nc.dram_tensor('tap_' + nm, list(t.shape),
                                   t.dtype, kind="ExternalOutput")
                nc.sync.dma_start(d[:], t[:])
            pools['tapfn'] = tapfn

            qs = [nc.sync, nc.scalar]

            def load_qk(pfx, l):
                wq = watp.tile([P, KO, DE], F8, tag='w_wq8')
                qs[0].dma_start(wq[:], din[f'{pfx}_wq8'][l])
                wk = watp.tile([P, KO, DE], F8, tag='w_wk8')
                qs[1].dma_start(wk[:], din[f'{pfx}_wk8'][l])
                return wq, wk

            # ======== encoder ========
            e16 = stream.tile([P, KO, LL], F16, tag='e16_a')
            nc.sync.dma_start(e16[:], z0l16[:])
            e8 = stream.tile([P, KO, LL], F8, tag='e8_a')
            nc.scalar.dma_start(e8[:], z0l8[:])
            Zfull = stream.tile([P, KO, L], F16, tag='Zfull')
            nc.sync.dma_start(Zfull[:], z0f16[:])
            Zfull8 = stream.tile([P, KO, L], F8, tag='Zfull8')
            nc.scalar.dma_start(Zfull8[:], z0f8[:])

            pres = presp.tile([P, KO, LL], F32, tag='pres')
            for l in range(LENC):
                wq, wk = load_qk('enc', l)
                _attn(nc, tc, pools, pres, e16, e8, Zfull8, Zfull,
                      wq, wk, din['enc_wvT'][l], din['enc_woT'][l],
                      None, qs, f'e{l}a')
                e16 = stream.tile([P, KO, LL], F16, tag='e16_b')
                e8 = stream.tile([P, KO, LL], F8, tag='e8_b')
                _ln(nc, tc, pools, pres, e16, e8, f'e{l}ln1')
                _mlp(nc, tc, pools, pres, e16, din['enc_w1T'][l],
                     din['enc_w2T'][l], qs, f'e{l}m')
                e16 = stream.tile([P, KO, LL], F16, tag='e16_a')
                e8 = stream.tile([P, KO, LL], F8, tag='e8_a')
                _ln(nc, tc, pools, pres, e16, e8, f'e{l}ln2')
                Zfull = stream.tile([P, KO, L], F16, tag='Zfull')
                Zfull8 = stream.tile([P, KO, L], F8, tag='Zfull8')
                _allgather_pair(nc, tc, pools, e16, Zfull, Zfull8,
                                agin, agout, f'e{l}')

            # ======== decoder ========
            e16 = stream.tile([P, KO, LL], F16, tag='e16_b')
            nc.sync.dma_start(e16[:], x0l16[:])
            e8 = stream.tile([P, KO, LL], F8, tag='e8_b')
            nc.scalar.dma_start(e8[:], x0l8[:])
            Xfull = stream.tile([P, KO, L], F16, tag='Xfull')
            nc.sync.dma_start(Xfull[:], x0f16[:])
            Xfull8 = stream.tile([P, KO, L], F8, tag='Xfull8')
            nc.scalar.dma_start(Xfull8[:], x0f8[:])

            for l in range(LDEC):
                wq, wk = load_qk('dec', l)
                _attn(nc, tc, pools, pres, e16, e8, Xfull8, Xfull,
                      wq, wk, din['dec_wvT'][l], din['dec_woT'][l],
                      msk16, qs, f'd{l}s')
                e16 = stream.tile([P, KO, LL], F16, tag='e16_a')
                e8 = stream.tile([P, KO, LL], F8, tag='e8_a')
                _ln(nc, tc, pools, pres, e16, e8, f'd{l}ln1')
                _attn(nc, tc, pools, pres, e16, e8, Zfull8, Zfull,
                      wq, wk, din['dec_wvT'][l], din['dec_woT'][l],
                      None, qs, f'd{l}c')
                e16 = stream.tile([P, KO, LL], F16, tag='e16_b')
                e8 = stream.tile([P, KO, LL], F8, tag='e8_b')
                _ln(nc, tc, pools, pres, e16, e8, f'd{l}ln2')
                _mlp(nc, tc, pools, pres, e16, din['dec_w1T'][l],
                     din['dec_w2T'][l], qs, f'd{l}m')
                if l < LDEC - 1:
                    e16 = stream.tile([P, KO, LL], F16, tag='e16_a')
                    e8 = stream.tile([P, KO, LL], F8, tag='e8_a')
                    _ln(nc, tc, pools, pres, e16, e8, f'd{l}ln3')
                    Xfull = stream.tile([P, KO, L], F16, tag='Xfull')
                    Xfull8 = stream.tile([P, KO, L], F8, tag='Xfull8')
                    _allgather_pair(nc, tc, pools, e16, Xfull, Xfull8,
                                    agin, agout, f'd{l}')
                else:
                    _ln(nc, tc, pools, pres, xu, None, f'd{l}ln3')

        # ========== unembed phase (position-sharded, no collectives) ========
        # Emits UNNORMALIZED exp(logits) per chunk (DMA'd out immediately,
        # overlapping the remaining matmuls) plus the per-position softmax
        # denominator; the host divides during assembly.
        with contextlib.ExitStack() as ctx:
            usb = ctx.enter_context(tc.tile_pool(name='usb', bufs=1))
            wup = ctx.enter_context(tc.tile_pool(name='wup', bufs=4))
            expp_ = ctx.enter_context(tc.tile_pool(name='expp_', bufs=4))
            up = ctx.enter_context(tc.tile_pool(name='up', bufs=6,
                                                space='PSUM'))
            dacc = usb.tile([P, 2, UNC], F32, tag='dacc')
            # dedicated queues: wut loads on scalar's hwdge, outputs on
            # sync's — avoids head-of-line blocking between them.
            for c in range(UNC):
                wut = wup.tile([P, KO, UC], F16, tag='wut')
                nc.sync.dma_start(wut[:], wuT[:, :, c * UC:(c + 1) * UC])
                for pt in range(2):
                    ps = up.tile([P, UC], F32, tag='ups', name=f'ups{c}{pt}')
                    for k in range(KO):
                        nc.tensor.matmul(ps[:], xu[:, k, pt * P:(pt + 1) * P],
                                         wut[:, k, :], start=(k == 0),
                                         stop=(k == KO - 1))
                    ex = expp_.tile([P, UC], F16, tag='uex')
                    nc.scalar.activation(ex[:], ps[:], AF.Exp,
                                         accum_out=dacc[:, pt, c:c + 1])
                    nc.scalar.dma_start(
                        outp[pt, :, c * UC:(c + 1) * UC], ex[:])
            deno = usb.tile([P, 2], F32, tag='deno')
            nc.vector.tensor_reduce(deno[:], dacc[:], mybir.AxisListType.X,
                                    ALU.add)
            nc.sync.dma_start(deno_d[:], deno[:])

    nc.compile()
    return nc


# ----------------------------------------------------------------------------
# host-side prep
# ----------------------------------------------------------------------------

def _to_kimaj(a):
    """[K, M] -> [128, K//128, M] with K = ko*128 + ki."""
    K, M = a.shape
    return np.ascontiguousarray(
        a.reshape(K // P, P, M).transpose(1, 0, 2))


def _fp8(a):
    return np.clip(a, -240.0, 240.0).astype(mybir.dt.np(F8))


def prep_inputs(inputs):
    f = lambda k: np.asarray(inputs[k], dtype=np.float32)
    We, Wp, Wu = f('We'), f('Wp'), f('Wu')
    x = np.asarray(inputs['x']).astype(np.int64)
    z = np.asarray(inputs['z']).astype(np.int64)

    shared = {}
    for pfx, nl in (('enc', LENC), ('dec', LDEC)):
        Wq, Wk, Wv = f(pfx + '_Wq'), f(pfx + '_Wk'), f(pfx + '_Wv')
        Wo, W1, W2 = f(pfx + '_Wo'), f(pfx + '_W1'), f(pfx + '_W2')
        wq, wk, wv, wo, w1, w2 = [], [], [], [], [], []
        for l in range(nl):
            qa = Wq[l].reshape(H * DA, DE).T * WS
            ka = Wk[l].reshape(H * DA, DE).T * WS
            va = Wv[l].transpose(2, 0, 1).reshape(DE, H * DA)
            wq.append(_to_kimaj(qa)); wk.append(_to_kimaj(ka))
            wv.append(_to_kimaj(va))
            wo.append(_to_kimaj(Wo[l].T))
            w1.append(_to_kimaj(W1[l].T))
            w2.append(_to_kimaj(W2[l].T))
        shared[f'{pfx}_wq8'] = _fp8(np.stack(wq))
        shared[f'{pfx}_wk8'] = _fp8(np.stack(wk))
        shared[f'{pfx}_wvT'] = np.stack(wv).astype(np.float16)
        shared[f'{pfx}_woT'] = np.stack(wo).astype(np.float16)
        shared[f'{pfx}_w1T'] = np.stack(w1).astype(np.float16)
        shared[f'{pfx}_w2T'] = np.stack(w2).astype(np.float16)

    shared['wuT'] = _to_kimaj(Wu.T).astype(np.float16)
    shared['diag1'] = np.eye(P, dtype=np.float16)

    pos = Wp[:L]  # [512, 1024]
    in_maps = []
    for c in range(N_CORES):
        b, h = c // 2, c % 2
        m = dict(shared)
        for nm, tok in (('z0', z[b]), ('x0', x[b])):
            E0 = (We[tok] + pos).T.astype(np.float32)      # [1024, 512]
            E0k = E0.reshape(KO, P, L)                     # [ko, ki, p]
            full = np.ascontiguousarray(E0k.transpose(1, 0, 2))
            m[nm + '_full16'] = full.astype(np.float16)
            m[nm + '_full8'] = _fp8(full)
            loc = np.ascontiguousarray(
                E0k[:, :, h * LL:(h + 1) * LL].transpose(1, 0, 2))
            m[nm + '_loc16'] = loc.astype(np.float16)
            m[nm + '_loc8'] = _fp8(loc)
        kglob = np.arange(L)[:, None]
        qglob = (h * LL + np.arange(LL))[None, :]
        msk = (kglob <= qglob).astype(np.float16)          # [512, 256]
        m['mask_self16'] = np.ascontiguousarray(
            msk.reshape(4, P, LL).transpose(1, 0, 2))
        in_maps.append(m)
    return in_maps


def assemble(results):
    """results: per-core dicts with 'outp' [2, 128, NV] fp16 (unnormalized
    exp of logits) and 'deno_out' [128, 2] f32 softmax denominators."""
    out = np.empty((4, NV, L), dtype=np.float32)
    for c, r in enumerate(results):
        b, h = c // 2, c % 2
        o = r['outp'].astype(np.float32).reshape(LL, NV)   # [pos, vocab]
        deno = r['deno_out'].T.reshape(LL, 1)              # [pos, 1]
        out[b, :, h * LL:(h + 1) * LL] = (o / deno).T
    return out


def run(inputs, trace=False, taps=(), trace_kwargs=None):
    key = ('prog', tuple(sorted(taps)))
    if key not in _CACHE:
        _CACHE[key] = build_program(taps=taps)
    nc = _CACHE[key]
    in_maps = prep_inputs(inputs)
    res = run_bass_kernel_spmd(nc, in_maps, list(range(N_CORES)),
                               trace=trace, **(trace_kwargs or {}))
    return res


def kernel(**inputs):
    res = run(inputs, trace=False)
    return assemble(res.results)


# revision 36
# speedup vs baseline: 1.3834x; 1.0630x over previous
"""Trainium2 Bass kernel for nn_EDTransformer (encoder-decoder transformer).

Sharding: 8 cores = 4 batch items x 2 sequence halves.
 - Each core owns (item b, half h): Q/scores/AV/Wo/MLP/LN for its 256 local
   positions, K/V redundantly for the full 512 positions.
 - One 2-core AllGather of fp16 activations per layer boundary.
 - Unembedding position-sharded (full vocab per core), softmax denominator
   via fused activation accumulate.

Precision plan:
 - Q/K projections in fp8e4 DoubleRow matmuls (2x PE rate): their
   quantization noise only jitters softmax scores, which averages out.
 - Everything on the value path (V fill, exp, AV, deno, y, Wo) plus MLP,
   LN and unembed stays fp16 with fp32 PSUM accumulation.
 - Residual adds are injected into the Wo/W2 PSUM by an identity matmul
   (frees the vector engine; PSUM zero-region seeded by even-dt start).
"""
import sys

sys.path.insert(0, '/opt/trn_rl_repo')
import numpy as np

import concourse.bacc as bacc
import concourse.tile as tile
import concourse.mybir as mybir
from concourse.bass_utils import run_bass_kernel_spmd

DT = mybir.dt
F16 = DT.float16
F32 = DT.float32
F8 = DT.float8e4
DR = mybir.MatmulPerfMode.DoubleRow
AF = mybir.ActivationFunctionType
ALU = mybir.AluOpType

N_CORES = 8
P = 128
DE = 1024          # model dim        (8 ptiles)
KO = DE // P       # 8
DMLP = 4096        # mlp dim          (32 ptiles)
MO = DMLP // P     # 32
H = 16             # heads
DA = 64            # attn dim per head
L = 512            # sequence length
LL = 256           # local positions per core
NV = 32000
UC = 500           # unembed vocab chunk (moving N)
UNC = NV // UC     # 64 chunks
LENC = 2
LDEC = 2
EPS = 1e-5

WS = 32.0                     # fp8 weight scale for Wq/Wk
EXPS = 1.0 / (WS * WS * 8.0)  # exp scale (folds q8*k8 scale + sqrt(da))

PAIR_GROUPS = [[0, 1], [2, 3], [4, 5], [6, 7]]

_CACHE = {}


# ----------------------------------------------------------------------------
# device program
# ----------------------------------------------------------------------------

def _attn(nc, tc, pools, pres, e16res, qin8, kv8, kv16, wq_t, wk_t,
          wv_dram, wo_dram, mask16, qs, name):
    """One multi-head attention block. Leaves pres [128, 8, 256] f32 psum
    holding attn_out + residual.

    qin8  : [128, KO, LL] fp8  local stream (query input)
    kv8   : [128, KO, L]  fp8  full-seq stream (key input)
    kv16  : [128, KO, L]  fp16 full-seq stream (value input)
    wq_t/wk_t : SBUF fp8 weight tiles [128, KO, 1024] (x32 scaled)
    wv_dram/wo_dram : fp16 dram APs [128, KO, 1024]
    mask16: [128, 4, LL] fp16 sbuf tile or None
    """
    sb = pools['att']
    pA = pools['pA']
    KT = L // P  # 4 kz tiles

    # ---- residual identity injection (start=True on even dt pre-zeroes
    # the 2KB psum zero-region covering the odd partner as well).
    for dt in range(KO):
        nc.tensor.matmul(pres[:, dt, :], pools['diag1'][:, :],
                         e16res[:, dt, :], start=(dt % 2 == 0), stop=False,
                         skip_group_check=True)

    # prefetch V weights (used after Q/K fills)
    wvts = []
    for nch in range(2):
        wvt = pools['wvp'].tile([P, KO, 512], F16, tag='wvt')
        qs[nch % 2].dma_start(wvt[:], wv_dram[:, :, nch * 512:(nch + 1) * 512])
        wvts.append(wvt)

    # ---- q16: [128(2h x 64a), pr, LL]  (fp8 DR matmul, fp16 result)
    q16 = sb.tile([P, KO, LL], F16, tag='q16')
    for pr in range(KO):
        ps = pA.tile([P, L], F32, tag='pA')
        for j in range(KO // 2):
            nc.tensor.matmul(ps[:, :LL], wq_t[:, 2 * j:2 * j + 2,
                                              pr * P:(pr + 1) * P],
                             qin8[:, 2 * j:2 * j + 2, :],
                             start=(j == 0), stop=(j == 3), perf_mode=DR)
        nc.vector.tensor_copy(q16[:, pr, :], ps[:, :LL])
    # ---- k16: [128, pr, L]
    k16 = sb.tile([P, KO, L], F16, tag='k16')
    for pr in range(KO):
        ps = pA.tile([P, L], F32, tag='pA')
        for j in range(KO // 2):
            nc.tensor.matmul(ps[:], wk_t[:, 2 * j:2 * j + 2,
                                         pr * P:(pr + 1) * P],
                             kv8[:, 2 * j:2 * j + 2, :],
                             start=(j == 0), stop=(j == 3), perf_mode=DR)
        nc.vector.tensor_copy(k16[:, pr, :], ps[:])
    # ---- vT: [128(kz), kt, 1024(h*64)] fp16
    vt16 = sb.tile([P, KT, H * DA], F16, tag='vt16')
    for nch in range(2):
        wvt = wvts[nch]
        for kt in range(KT):
            ps = pA.tile([P, L], F32, tag='pA')
            for k in range(KO):
                nc.tensor.matmul(ps[:], kv16[:, k, kt * P:(kt + 1) * P],
                                 wvt[:, k, :],
                                 start=(k == 0), stop=(k == KO - 1))
            nc.vector.tensor_copy(vt16[:, kt, nch * 512:(nch + 1) * 512],
                                  ps[:])

    # prefetch Wo chunks (consumed after the head loop)
    wots = []
    for c in range(4):
        wot = pools['wop'].tile([P, KO, 2 * P], F16, tag='wot')
        qs[c % 2].dma_start(wot[:], wo_dram[:, :, c * 2 * P:(c + 1) * 2 * P])
        wots.append(wot)

    # ---- per head pair: scores -> exp -> deno -> AV -> y16.
    # Software-pipelined by one pair: scores/exp for pair p are issued
    # before deno/AV of pair p-1 so the tensor engine (in-order) never
    # blocks on the scalar exp of the pair it is about to reduce.
    y16 = sb.tile([P, KO, LL], F16, tag='y16')
    expps = {}

    def emit_scores(p):
        hA = 2 * p
        expp = sb.tile([P, KT, 2, LL], F16, tag='expp', bufs=2)
        expps[p] = expp
        for hh in range(2):
            h = hA + hh
            pr, hp = h // 2, (h % 2) * DA
            for t in range(2):
                ps = pA.tile([P, L], F32, tag='pA')
                for kt in (2 * t, 2 * t + 1):
                    nc.tensor.matmul(
                        ps[:, (kt - 2 * t) * LL:(kt - 2 * t + 1) * LL],
                        k16[hp:hp + DA, pr, kt * P:(kt + 1) * P],
                        q16[hp:hp + DA, pr, :],
                        start=True, stop=True)
                nc.scalar.activation(
                    expp[:, 2 * t:2 * t + 2, hh, :],
                    ps[:].rearrange('p (a b) -> p a b', a=2),
                    AF.Exp, scale=EXPS)
            if mask16 is not None:
                nc.vector.tensor_tensor(
                    expp[:, :, hh, :], expp[:, :, hh, :],
                    mask16[:], ALU.mult)

    def emit_reduce(p):
        hA = 2 * p
        expp = expps.pop(p)
        # deno: ones-matmul -> [128, (hh, q)]
        pd = pA.tile([P, L], F32, tag='pA')
        for kt in range(KT):
            nc.tensor.matmul(pd[:], pools['ones16'][:, :],
                             expp[:, kt, :, :],
                             start=(kt == 0), stop=(kt == KT - 1))
        ysc = pools['yscp'].tile([P, 2 * LL], F32, tag='ysc')
        nc.vector.reciprocal_approx_fast(ysc[:], pd[:])
        # AV, pair-packed via column tiling
        pav = pA.tile([P, L], F32, tag='pA')
        for hh in range(2):
            hcol = (hA + hh) * DA
            for kt in range(KT):
                nc.tensor.matmul(
                    pav[hh * DA:(hh + 1) * DA, :LL],
                    vt16[:, kt, hcol:hcol + DA],
                    expp[:, kt, hh, :],
                    start=(kt == 0), stop=(kt == KT - 1),
                    tile_position=(0, hh * DA))
        nc.vector.tensor_tensor(y16[:DA, p, :], pav[:DA, :LL],
                                ysc[:DA, 0:LL], ALU.mult)
        nc.vector.tensor_tensor(y16[DA:, p, :], pav[DA:, :LL],
                                ysc[DA:, LL:2 * LL], ALU.mult)

    emit_scores(0)
    for p in range(1, KO):
        emit_scores(p)
        emit_reduce(p - 1)
    emit_reduce(KO - 1)

    # ---- Wo accumulate into pres (start=False: id-matmul zeroed/seeded)
    for c in range(4):
        wot = wots[c]
        for di in range(2):
            dt = 2 * c + di
            for k in range(KO):
                nc.tensor.matmul(pres[:, dt, :],
                                 wot[:, k, di * P:(di + 1) * P],
                                 y16[:, k, :],
                                 start=False, stop=(k == KO - 1),
                                 skip_group_check=True)
    tp = pools.get('tapfn')
    if tp:
        tp(f'{name}_q', q16); tp(f'{name}_k', k16); tp(f'{name}_vt', vt16)
        tp(f'{name}_y', y16)


def _mlp(nc, tc, pools, pres, e16res, w1_dram, w2_dram, qs, name):
    """MLP block (fp16). Leaves pres holding mlp_out + residual."""
    pA = pools['pA']
    for dt in range(KO):
        nc.tensor.matmul(pres[:, dt, :], pools['diag1'][:, :],
                         e16res[:, dt, :], start=(dt % 2 == 0), stop=False,
                         skip_group_check=True)
    h16 = pools['mlp'].tile([P, MO, LL], F16, tag='h16')
    for c in range(8):
        w1t = pools['w1p'].tile([P, KO, 4 * P], F16, tag='w1t')
        qs[c % 2].dma_start(w1t[:], w1_dram[:, :, c * 512:(c + 1) * 512])
        for m2 in range(2):   # 2 mt per psum tile
            ps = pA.tile([P, L], F32, tag='pA')
            for mi in range(2):
                for k in range(KO):
                    nc.tensor.matmul(ps[:, mi * LL:(mi + 1) * LL],
                                     w1t[:, k, (2 * m2 + mi) * P:
                                         (2 * m2 + mi + 1) * P],
                                     e16res[:, k, :],
                                     start=(k == 0), stop=(k == KO - 1))
            nc.scalar.activation(h16[:, 4 * c + 2 * m2:4 * c + 2 * m2 + 2, :],
                                 ps[:].rearrange('p (a b) -> p a b', a=2),
                                 AF.Relu)
    for c in range(8):
        w2t = pools['w2p'].tile([P, 4, KO * P], F16, tag='w2t')
        qs[c % 2].dma_start(w2t[:], w2_dram[:, c * 4:(c + 1) * 4, :])
        for j in range(4):
            for dt in range(KO):
                nc.tensor.matmul(pres[:, dt, :],
                                 w2t[:, j, dt * P:(dt + 1) * P],
                                 h16[:, c * 4 + j, :],
                                 start=False,
                                 stop=(c == 7 and j == 3),
                                 skip_group_check=True)


def _ln(nc, tc, pools, pres, e16out, e8out, name):
    """Layernorm from psum pres -> fp16 (+fp8) stream."""
    pA = pools['pA']
    lnp = pools['lnp']
    stat = pools['stat']
    pre16 = lnp.tile([P, KO, LL], F16, tag='pre16')
    nc.scalar.activation(pre16[:], pres[:], AF.Copy)
    sq16 = lnp.tile([P, KO, LL], F16, tag='sq16')
    nc.vector.tensor_tensor(sq16[:], pre16[:], pre16[:], ALU.mult)
    pss = pA.tile([P, L], F32, tag='pA')
    for k in range(KO):
        nc.tensor.matmul(pss[:, :LL], pools['ones16'][:, :], pre16[:, k, :],
                         start=(k == 0), stop=(k == KO - 1))
    psq = pA.tile([P, L], F32, tag='pA')
    for k in range(KO):
        nc.tensor.matmul(psq[:, :LL], pools['ones16'][:, :], sq16[:, k, :],
                         start=(k == 0), stop=(k == KO - 1))
    # v1 = Q - S^2/1024 (= 1023*var); inv = 1/sqrt(v1/1023 + eps);
    # nm = -S/1024 * inv
    s2 = stat.tile([P, LL], F32, tag='s2')
    nc.scalar.activation(s2[:], pss[:, :LL], AF.Square)
    v1 = stat.tile([P, LL], F32, tag='v1')
    nc.vector.scalar_tensor_tensor(v1[:], s2[:], -1.0 / 1024.0, psq[:, :LL],
                                   ALU.mult, ALU.add)
    std = stat.tile([P, LL], F32, tag='std')
    nc.scalar.activation(std[:], v1[:], AF.Sqrt, bias=pools['eps128'],
                         scale=1.0 / 1023.0)
    inv = stat.tile([P, LL], F32, tag='inv')
    nc.vector.reciprocal_approx_fast(inv[:], std[:])
    nm = stat.tile([P, LL], F32, tag='nm')
    nc.vector.scalar_tensor_tensor(nm[:], pss[:, :LL], -1.0 / 1024.0, inv[:],
                                   ALU.mult, ALU.mult)
    nc.vector.tensor_tensor(
        e16out[:], pre16[:],
        inv[:, None, :].to_broadcast((P, KO, LL)), ALU.mult)
    nc.vector.tensor_tensor(
        e16out[:], e16out[:],
        nm[:, None, :].to_broadcast((P, KO, LL)), ALU.add)
    if e8out is not None:
        nc.vector.tensor_copy(e8out[:], e16out[:])
    tp = pools.get('tapfn')
    if tp:
        tp(f'{name}_out', e16out)


def _allgather_pair(nc, tc, pools, e16loc, full16, full8, agin, agout, tag):
    """e16loc [128, KO, LL] -> pair AllGather -> full16 (+full8)."""
    nc.scalar.dma_start(agin[:], e16loc[:])
    nc.gpsimd.collective_compute(
        "AllGather", ALU.bypass,
        ins=[agin[:]], outs=[agout[:]],
        replica_groups=PAIR_GROUPS)
    nc.sync.dma_start(
        full16[:].rearrange('ki ko (r p) -> ki ko r p', r=2),
        agout[:].rearrange('r ki ko p -> ki ko r p'))
    if full8 is not None:
        nc.vector.tensor_copy(full8[:], full16[:])


def build_program(taps=()):
    taps = set(taps)
    nc = bacc.Bacc("TRN2", target_bir_lowering=False, debug=False,
                   num_devices=N_CORES)

    # ---- dram inputs ----
    din = {}
    def dram_in(nm, shape, dt=F16):
        din[nm] = nc.dram_tensor(nm, list(shape), dt, kind="ExternalInput")
        return din[nm]

    z0f16 = dram_in('z0_full16', [P, KO, L])
    x0f16 = dram_in('x0_full16', [P, KO, L])
    z0f8 = dram_in('z0_full8', [P, KO, L], F8)
    x0f8 = dram_in('x0_full8', [P, KO, L], F8)
    z0l8 = dram_in('z0_loc8', [P, KO, LL], F8)
    x0l8 = dram_in('x0_loc8', [P, KO, LL], F8)
    z0l16 = dram_in('z0_loc16', [P, KO, LL])
    x0l16 = dram_in('x0_loc16', [P, KO, LL])
    mask_self = dram_in('mask_self16', [P, 4, LL])
    diag1_d = dram_in('diag1', [P, P])
    for pfx, nl in (('enc', LENC), ('dec', LDEC)):
        for w in ('wq8', 'wk8'):
            dram_in(f'{pfx}_{w}', [nl, P, KO, DE], F8)
        for w in ('wvT', 'woT'):
            dram_in(f'{pfx}_{w}', [nl, P, KO, DE])
        dram_in(f'{pfx}_w1T', [nl, P, KO, DMLP])
        dram_in(f'{pfx}_w2T', [nl, P, MO, DE])
    # per-core vocab half (pair vocab-sharding of the unembedding)
    NVH = NV // 2
    wuT = dram_in('wuT', [P, KO, NVH])

    # output: [pt, ki, vocab-half] fp16 UNNORMALIZED exp(logits) for ALL
    # 512 positions (pos = pt*128 + ki); deno_d = PARTIAL softmax
    # denominators over this core's vocab half (host sums the pair).
    outp = nc.dram_tensor('outp', [4, P, NVH], F16, kind="ExternalOutput")
    deno_d = nc.dram_tensor('deno_out', [P, 4], F32, kind="ExternalOutput")

    # internal dram for pair collectives
    agin = nc.dram_tensor('agin', [P, KO, LL], F16)
    agout = nc.dram_tensor('agout', [2, P, KO, LL], F16)
    wuin = nc.dram_tensor('wuin', [1, 64], F16)
    wuout = nc.dram_tensor('wuout', [2, 64], F16)

    import contextlib
    with tile.TileContext(nc) as tc, contextlib.ExitStack() as octx:
        const = octx.enter_context(tc.tile_pool(name='const', bufs=1))
        ones16 = const.tile([P, P], F16)
        nc.vector.memset(ones16[:], 1.0)
        eps128 = const.tile([P, 1], F32)
        nc.vector.memset(eps128[:], EPS)
        diag1 = const.tile([P, P], F16)
        nc.sync.dma_start(diag1[:], diag1_d[:])
        msk16 = const.tile([P, 4, LL], F16)
        nc.sync.dma_start(msk16[:], mask_self[:])
        xu = const.tile([P, KO, L], F16)   # full 512 positions (gathered)
        # warmup collective: absorbs the cc-infrastructure spin-up (~40us)
        # off the critical path before the first real AllGather.
        wut0 = const.tile([1, 64], F16)
        nc.vector.memset(wut0[:], 0.0)
        nc.gpsimd.dma_start(wuin[:], wut0[:])
        nc.gpsimd.collective_compute(
            "AllGather", ALU.bypass, ins=[wuin[:]], outs=[wuout[:]],
            replica_groups=PAIR_GROUPS)

        # ================= layer phase =================
        with contextlib.ExitStack() as ctx:
            stream = ctx.enter_context(tc.tile_pool(name='stream', bufs=1))
            att = ctx.enter_context(tc.tile_pool(name='att', bufs=1))
            mlpp = ctx.enter_context(tc.tile_pool(name='mlpp', bufs=1))
            lnp = ctx.enter_context(tc.tile_pool(name='lnp', bufs=1))
            stat = ctx.enter_context(tc.tile_pool(name='stat', bufs=1))
            yscp = ctx.enter_context(tc.tile_pool(name='yscp', bufs=2))
            watp = ctx.enter_context(tc.tile_pool(name='watp', bufs=1))
            wvp = ctx.enter_context(tc.tile_pool(name='wvp', bufs=2))
            wop = ctx.enter_context(tc.tile_pool(name='wop', bufs=4))
            w1p = ctx.enter_context(tc.tile_pool(name='w1p', bufs=3))
            w2p = ctx.enter_context(tc.tile_pool(name='w2p', bufs=3))
            pA = ctx.enter_context(tc.tile_pool(name='pA', bufs=4,
                                                space='PSUM'))
            presp = ctx.enter_context(tc.tile_pool(name='presp', bufs=1,
                                                   space='PSUM'))

            pools = dict(att=att, mlp=mlpp, lnp=lnp, stat=stat, yscp=yscp,
                         pA=pA, wvp=wvp, wop=wop, w1p=w1p, w2p=w2p,
                         ones16=ones16, eps128=eps128[:], diag1=diag1)

            def tapfn(nm, t):
                if nm not in taps:
                    return
                d = nc.dram_tensor('tap_' + nm, list(t.shape),
                                   t.dtype, kind="ExternalOutput")
                nc.sync.dma_start(d[:], t[:])
            pools['tapfn'] = tapfn

            qs = [nc.sync, nc.scalar]

            def load_qk(pfx, l):
                wq = watp.tile([P, KO, DE], F8, tag='w_wq8')
                qs[0].dma_start(wq[:], din[f'{pfx}_wq8'][l])
                wk = watp.tile([P, KO, DE], F8, tag='w_wk8')
                qs[1].dma_start(wk[:], din[f'{pfx}_wk8'][l])
                return wq, wk

            # ======== encoder ========
            e16 = stream.tile([P, KO, LL], F16, tag='e16_a')
            nc.sync.dma_start(e16[:], z0l16[:])
            e8 = stream.tile([P, KO, LL], F8, tag='e8_a')
            nc.scalar.dma_start(e8[:], z0l8[:])
            Zfull = stream.tile([P, KO, L], F16, tag='Zfull')
            nc.sync.dma_start(Zfull[:], z0f16[:])
            Zfull8 = stream.tile([P, KO, L], F8, tag='Zfull8')
            nc.scalar.dma_start(Zfull8[:], z0f8[:])

            pres = presp.tile([P, KO, LL], F32, tag='pres')
            for l in range(LENC):
                wq, wk = load_qk('enc', l)
                _attn(nc, tc, pools, pres, e16, e8, Zfull8, Zfull,
                      wq, wk, din['enc_wvT'][l], din['enc_woT'][l],
                      None, qs, f'e{l}a')
                e16 = stream.tile([P, KO, LL], F16, tag='e16_b')
                e8 = stream.tile([P, KO, LL], F8, tag='e8_b')
                _ln(nc, tc, pools, pres, e16, e8, f'e{l}ln1')
                _mlp(nc, tc, pools, pres, e16, din['enc_w1T'][l],
                     din['enc_w2T'][l], qs, f'e{l}m')
                e16 = stream.tile([P, KO, LL], F16, tag='e16_a')
                e8 = stream.tile([P, KO, LL], F8, tag='e8_a')
                _ln(nc, tc, pools, pres, e16, e8, f'e{l}ln2')
                Zfull = stream.tile([P, KO, L], F16, tag='Zfull')
                Zfull8 = stream.tile([P, KO, L], F8, tag='Zfull8')
                _allgather_pair(nc, tc, pools, e16, Zfull, Zfull8,
                                agin, agout, f'e{l}')

            # ======== decoder ========
            e16 = stream.tile([P, KO, LL], F16, tag='e16_b')
            nc.sync.dma_start(e16[:], x0l16[:])
            e8 = stream.tile([P, KO, LL], F8, tag='e8_b')
            nc.scalar.dma_start(e8[:], x0l8[:])
            Xfull = stream.tile([P, KO, L], F16, tag='Xfull')
            nc.sync.dma_start(Xfull[:], x0f16[:])
            Xfull8 = stream.tile([P, KO, L], F8, tag='Xfull8')
            nc.scalar.dma_start(Xfull8[:], x0f8[:])

            for l in range(LDEC):
                wq, wk = load_qk('dec', l)
                _attn(nc, tc, pools, pres, e16, e8, Xfull8, Xfull,
                      wq, wk, din['dec_wvT'][l], din['dec_woT'][l],
                      msk16, qs, f'd{l}s')
                e16 = stream.tile([P, KO, LL], F16, tag='e16_a')
                e8 = stream.tile([P, KO, LL], F8, tag='e8_a')
                _ln(nc, tc, pools, pres, e16, e8, f'd{l}ln1')
                _attn(nc, tc, pools, pres, e16, e8, Zfull8, Zfull,
                      wq, wk, din['dec_wvT'][l], din['dec_woT'][l],
                      None, qs, f'd{l}c')
                e16 = stream.tile([P, KO, LL], F16, tag='e16_b')
                e8 = stream.tile([P, KO, LL], F8, tag='e8_b')
                _ln(nc, tc, pools, pres, e16, e8, f'd{l}ln2')
                _mlp(nc, tc, pools, pres, e16, din['dec_w1T'][l],
                     din['dec_w2T'][l], qs, f'd{l}m')
                if l < LDEC - 1:
                    e16 = stream.tile([P, KO, LL], F16, tag='e16_a')
                    e8 = stream.tile([P, KO, LL], F8, tag='e8_a')
                    _ln(nc, tc, pools, pres, e16, e8, f'd{l}ln3')
                    Xfull = stream.tile([P, KO, L], F16, tag='Xfull')
                    Xfull8 = stream.tile([P, KO, L], F8, tag='Xfull8')
                    _allgather_pair(nc, tc, pools, e16, Xfull, Xfull8,
                                    agin, agout, f'd{l}')
                else:
                    xuloc = stream.tile([P, KO, LL], F16, tag='e16_a')
                    _ln(nc, tc, pools, pres, xuloc, None, f'd{l}ln3')
                    _allgather_pair(nc, tc, pools, xuloc, xu, None,
                                    agin, agout, f'd{l}xu')

        # ========== unembed phase (position-sharded, no collectives) ========
        # Emits UNNORMALIZED exp(logits) per chunk (DMA'd out immediately,
        # overlapping the remaining matmuls) plus the per-position softmax
        # denominator; the host divides during assembly.
        with contextlib.ExitStack() as ctx:
            usb = ctx.enter_context(tc.tile_pool(name='usb', bufs=1))
            wup = ctx.enter_context(tc.tile_pool(name='wup', bufs=4))
            expp_ = ctx.enter_context(tc.tile_pool(name='expp_', bufs=4))
            up = ctx.enter_context(tc.tile_pool(name='up', bufs=6,
                                                space='PSUM'))
            UNCH = UNC // 2   # 32 chunks over this core's vocab half
            dacc = usb.tile([P, 4, UNCH], F32, tag='dacc')
            # dedicated queues: wut loads on sync's hwdge, exp outputs on
            # scalar's — avoids head-of-line blocking between them.
            for c in range(UNCH):
                wut = wup.tile([P, KO, UC], F16, tag='wut')
                nc.sync.dma_start(wut[:], wuT[:, :, c * UC:(c + 1) * UC])
                for pt in range(4):
                    ps = up.tile([P, UC], F32, tag='ups', name=f'ups{c}{pt}')
                    for k in range(KO):
                        nc.tensor.matmul(ps[:], xu[:, k, pt * P:(pt + 1) * P],
                                         wut[:, k, :], start=(k == 0),
                                         stop=(k == KO - 1))
                    ex = expp_.tile([P, UC], F16, tag='uex')
                    nc.scalar.activation(ex[:], ps[:], AF.Exp,
                                         accum_out=dacc[:, pt, c:c + 1])
                    nc.scalar.dma_start(
                        outp[pt, :, c * UC:(c + 1) * UC], ex[:])
            deno = usb.tile([P, 4], F32, tag='deno')
            nc.vector.tensor_reduce(deno[:], dacc[:], mybir.AxisListType.X,
                                    ALU.add)
            nc.sync.dma_start(deno_d[:], deno[:])

    nc.compile()
    return nc


# ----------------------------------------------------------------------------
# host-side prep
# ----------------------------------------------------------------------------

def _to_kimaj(a):
    """[K, M] -> [128, K//128, M] with K = ko*128 + ki."""
    K, M = a.shape
    return np.ascontiguousarray(
        a.reshape(K // P, P, M).transpose(1, 0, 2))


def _fp8(a):
    return np.clip(a, -240.0, 240.0).astype(mybir.dt.np(F8))


def prep_inputs(inputs):
    f = lambda k: np.asarray(inputs[k], dtype=np.float32)
    We, Wp, Wu = f('We'), f('Wp'), f('Wu')
    x = np.asarray(inputs['x']).astype(np.int64)
    z = np.asarray(inputs['z']).astype(np.int64)

    shared = {}
    for pfx, nl in (('enc', LENC), ('dec', LDEC)):
        Wq, Wk, Wv = f(pfx + '_Wq'), f(pfx + '_Wk'), f(pfx + '_Wv')
        Wo, W1, W2 = f(pfx + '_Wo'), f(pfx + '_W1'), f(pfx + '_W2')
        wq, wk, wv, wo, w1, w2 = [], [], [], [], [], []
        for l in range(nl):
            qa = Wq[l].reshape(H * DA, DE).T * WS
            ka = Wk[l].reshape(H * DA, DE).T * WS
            va = Wv[l].transpose(2, 0, 1).reshape(DE, H * DA)
            wq.append(_to_kimaj(qa)); wk.append(_to_kimaj(ka))
            wv.append(_to_kimaj(va))
            wo.append(_to_kimaj(Wo[l].T))
            w1.append(_to_kimaj(W1[l].T))
            w2.append(_to_kimaj(W2[l].T))
        shared[f'{pfx}_wq8'] = _fp8(np.stack(wq))
        shared[f'{pfx}_wk8'] = _fp8(np.stack(wk))
        shared[f'{pfx}_wvT'] = np.stack(wv).astype(np.float16)
        shared[f'{pfx}_woT'] = np.stack(wo).astype(np.float16)
        shared[f'{pfx}_w1T'] = np.stack(w1).astype(np.float16)
        shared[f'{pfx}_w2T'] = np.stack(w2).astype(np.float16)

    wuk = _to_kimaj(Wu.T).astype(np.float16)
    shared['diag1'] = np.eye(P, dtype=np.float16)

    pos = Wp[:L]  # [512, 1024]
    in_maps = []
    NVH = NV // 2
    for c in range(N_CORES):
        b, h = c // 2, c % 2
        m = dict(shared)
        m['wuT'] = np.ascontiguousarray(wuk[:, :, h * NVH:(h + 1) * NVH])
        for nm, tok in (('z0', z[b]), ('x0', x[b])):
            E0 = (We[tok] + pos).T.astype(np.float32)      # [1024, 512]
            E0k = E0.reshape(KO, P, L)                     # [ko, ki, p]
            full = np.ascontiguousarray(E0k.transpose(1, 0, 2))
            m[nm + '_full16'] = full.astype(np.float16)
            m[nm + '_full8'] = _fp8(full)
            loc = np.ascontiguousarray(
                E0k[:, :, h * LL:(h + 1) * LL].transpose(1, 0, 2))
            m[nm + '_loc16'] = loc.astype(np.float16)
            m[nm + '_loc8'] = _fp8(loc)
        kglob = np.arange(L)[:, None]
        qglob = (h * LL + np.arange(LL))[None, :]
        msk = (kglob <= qglob).astype(np.float16)          # [512, 256]
        m['mask_self16'] = np.ascontiguousarray(
            msk.reshape(4, P, LL).transpose(1, 0, 2))
        in_maps.append(m)
    return in_maps


def assemble(results):
    """results: per-core dicts with 'outp' [4, 128, NV/2] fp16 (unnormalized
    exp of logits for this core's vocab half, all 512 positions) and
    'deno_out' [128, 4] f32 partial softmax denominators."""
    NVH = NV // 2
    out = np.empty((4, NV, L), dtype=np.float32)
    for b in range(4):
        r0, r1 = results[2 * b], results[2 * b + 1]
        deno = (r0['deno_out'].T.reshape(L) +
                r1['deno_out'].T.reshape(L))               # [pos]
        for h, r in ((0, r0), (1, r1)):
            o = r['outp'].astype(np.float32).reshape(L, NVH)
            out[b, h * NVH:(h + 1) * NVH, :] = (o / deno[:, None]).T
    return out


def run(inputs, trace=False, taps=(), trace_kwargs=None):
    key = ('prog', tuple(sorted(taps)))
    if key not in _CACHE:
        _CACHE[key] = build_program(taps=taps)
    nc = _CACHE[key]
    in_maps = prep_inputs(inputs)
    res = run_bass_kernel_spmd(nc, in_maps, list(range(N_CORES)),
                               trace=trace, **(trace_kwargs or {}))
    return res


def kernel(**inputs):
    res = run(inputs, trace=False)
    return assemble(res.results)
